# revision 36
# baseline (speedup 1.0000x reference)
"""CSR Linear kernel for TRN2: out = x @ W^T + bias, W from COO nonzeros.

Strategy: data-parallel over tokens across 8 NeuronCores. Host densifies the
sparse weight into A[in, out] (duplicate coords summed) in bf16; each core
computes its 1024-token shard as out^T = A^T-tiles stationary on the PE with
x^T streaming:  psum[128 outf, 512 tok] += A_tile[128 k, 128 outf].T @
xT[128 k, 512 tok].  With out-features on PSUM partitions the bias add is a
per-partition tensor_scalar on the eviction. bf16 operands halve DMA bytes
and enable fast weight load; phase A runs the first 4 out-tiles k-outer so
DMA demand stays under the per-core HBM rate from the first matmul, phase B
runs o-major k-sweeps at pure PE rate with per-sweep evictions.
"""

import os
import sys
import types

import ml_dtypes
import numpy as np

TOKENS = 8192
IN_F = 4096
OUT_F = 4096
N_CORES = 8
P = 128

_CACHE = {}


def _ensure_ntff_hook():
    """Register the axon NTFF profile hook if the antenv stub lacks it.

    Only needed when tracing (BASS_TRACE=1); harmless otherwise. In
    environments with a real antenv.axon_hooks this is a no-op.
    """
    try:
        import antenv.axon_hooks  # noqa: F401

        return
    except ImportError:
        pass
    try:
        import antenv
        from trn_agent_boot.trn_boot import _ntff_profile_via_ctypes

        hooks = types.ModuleType("antenv.axon_hooks")
        hooks._hook = _ntff_profile_via_ctypes("/opt/axon/libaxon_pjrt.so")
        hooks.set_axon_ntff_profile_hook = lambda h: setattr(hooks, "_hook", h)
        hooks.get_axon_ntff_profile_hook = lambda: hooks._hook
        sys.modules["antenv.axon_hooks"] = hooks
        antenv.axon_hooks = hooks
    except Exception:
        pass


def _patch_upload():
    """Make trace artifact upload fall back to the local tmpdir when no
    artifact bucket is reachable (container environments)."""
    from concourse import bass_utils

    orig = bass_utils.upload_artifacts
    if getattr(orig, "_kernel_patched", False):
        return

    def _safe_upload(tmpdir):
        try:
            return orig(tmpdir)
        except Exception:
            return tmpdir

    _safe_upload._kernel_patched = True
    bass_utils.upload_artifacts = _safe_upload


def build_program(tok_per_core=TOKENS // N_CORES, in_f=IN_F, out_f=OUT_F):
    """Build + compile the per-core Bass program.

    outT[out_f, tok_per_core] = sum_k A[k, :].T-tiles @ xT[k, tokens] + bias
    with A [in_f, out_f] bf16 (host-densified W^T), xT [in_f, tok] bf16.
    """
    key = (tok_per_core, in_f, out_f)
    if key in _CACHE:
        return _CACHE[key]

    import concourse.bacc as bacc
    import concourse.mybir as mybir
    import concourse.tile as tile

    KO = in_f // P  # 32 contraction tiles
    NB = out_f // P  # 32 out-feature tiles
    NH = tok_per_core // 512  # 2 token halves (psum bank = 512 f32)
    A_TILES = 4  # phase-A out-tiles (k-outer), 4*NH = 8 psum banks

    nc = bacc.Bacc("TRN2", target_bir_lowering=False, debug=False)
    # All DMAs in this kernel issue from the sync engine; drop the unused
    # scalar-engine HWDGE queue family so the NEFF declares (and tears down)
    # 16 fewer queues at exit.
    nc.m.queues = [q for q in nc.m.queues if q.name != "qScalarDynamicHW"]

    # xt2[p, ko*T + t] = x_shard^T[ko*128+p, t]
    xt = nc.dram_tensor("xt", [P, KO * tok_per_core], mybir.dt.bfloat16, kind="ExternalInput")
    # wt2[nb*128+p, ko*128+o] = A[ko*128+p, nb*128+o]
    wt = nc.dram_tensor("wt", [out_f, in_f], mybir.dt.bfloat16, kind="ExternalInput")
    # biasr[p, nb] = bias[nb*128+p]
    biasr = nc.dram_tensor("biasr", [P, NB], mybir.dt.float32, kind="ExternalInput")
    # outT[nb*128+p, t] = out[t, nb*128+p]; bf16 (host upcasts) — halves the
    # eviction DVE time and the output DMA bytes, ~0.17% added rounding.
    out = nc.dram_tensor("out", [out_f, tok_per_core], mybir.dt.bfloat16, kind="ExternalOutput")

    xt_ap = xt.ap().rearrange("p (ko t) -> p ko t", ko=KO)
    wt_ap = wt.ap().rearrange("(nb p) (ko o) -> p nb ko o", p=P, o=P)
    out_ap = out.ap().rearrange("(nb p) t -> p nb t", p=P)

    with tile.TileContext(nc) as tc:
        with (
            tc.tile_pool(name="xt_pool", bufs=1) as xt_pool,
            tc.tile_pool(name="warm_pool", bufs=1) as warm_pool,
            tc.tile_pool(name="bias_pool", bufs=1) as bias_pool,
            tc.tile_pool(name="wt_pool", bufs=5) as wt_pool,
            tc.tile_pool(name="out_pool", bufs=4) as out_pool,
            tc.tile_pool(name="psum", bufs=8, space="PSUM") as psum_pool,
        ):
            xt_sb = xt_pool.tile([P, KO, tok_per_core], mybir.dt.bfloat16)
            bias_sb = bias_pool.tile([P, NB], mybir.dt.float32)

            wt_tiles = {}

            def wt_tile(o):
                if o not in wt_tiles:
                    wt_tiles[o] = wt_pool.tile(
                        [P, KO, P], mybir.dt.bfloat16, name=f"wt_{o}", tag="wt"
                    )
                return wt_tiles[o]

            def load_wt(o, kb, kbe):
                nc.sync.dma_start(wt_tile(o)[:, kb:kbe, :], wt_ap[:, o, kb:kbe, :])

            def load_xt(kb, kbe):
                nc.sync.dma_start(xt_sb[:, kb:kbe, :], xt_ap[:, kb:kbe, :])

            def evict(o, ps, cb, cbe):
                ot = out_pool.tile(
                    [P, cbe - cb], mybir.dt.bfloat16, name=f"ot_{o}_{cb}", tag="ot"
                )
                nc.vector.tensor_scalar_add(ot[:], ps[:], bias_sb[:, o : o + 1])
                nc.sync.dma_start(out_ap[:, o, cb:cbe], ot[:])



            # ---- PE pre-warm: dummy matmuls on scratch during the DMA
            # startup hole so HAM un-throttles (K=8/8) before the first real
            # matmul instead of ~5us into phase A.
            warm_x = warm_pool.tile([P, 512], mybir.dt.bfloat16, name="warm_x")
            nc.vector.memset(warm_x[:], 0.0)
            warm_ps = psum_pool.tile([P, 512], mybir.dt.float32, name="warm_ps", tag="ps")
            for _ in range(6):
                nc.tensor.matmul(
                    warm_ps[:], lhsT=warm_x[:, 0:P], rhs=warm_x[:], start=True, stop=True
                )

            # ---- Phase A: out-tiles 0..3, k-outer so DMA demand is smooth ----
            # chunks sized fine at the start so the first matmul gates on
            # ~0.3 MiB of DMA, coarser later. xt rides finer-grained DMA
            # pieces than the MM chunking so matmuls gate on small arrivals.
            chunks = [(0, 1), (1, 4), (4, 8), (8, 16), (16, 24), (24, 32)]
            xt_pieces = [(0, 1), (1, 2), (2, 4), (4, 6), (6, 8)] + [
                (b, b + 2) for b in range(8, KO, 2)
            ]
            ps_a = {
                (o, h): psum_pool.tile([P, 512], mybir.dt.float32, name=f"psA_{o}_{h}", tag="ps")
                for o in range(A_TILES)
                for h in range(NH)
            }
            for ci, (kb, kbe) in enumerate(chunks):
                for pb, pbe in xt_pieces:
                    if pb >= kb and pbe <= kbe:
                        load_xt(pb, pbe)
                for o in range(A_TILES):
                    load_wt(o, kb, kbe)
                if ci == 2:
                    nc.sync.dma_start(bias_sb[:], biasr.ap())
                if ci == len(chunks) - 2:
                    load_wt(A_TILES, 0, KO)  # phase-B prefetch into spare bufs
                if ci == len(chunks) - 1:
                    load_wt(A_TILES + 1, 0, KO)
            for kb, kbe in chunks:
                for o in range(A_TILES):
                    wto = wt_tile(o)
                    for ko in range(kb, kbe):
                        for h in range(NH):
                            nc.tensor.matmul(
                                ps_a[(o, h)][:],
                                lhsT=wto[:, ko, :],
                                rhs=xt_sb[:, ko, h * 512 : (h + 1) * 512],
                                start=(ko == 0),
                                stop=(ko == KO - 1),
                            )
            for o in range(A_TILES):
                for h in range(NH):
                    evict(o, ps_a[(o, h)], h * 512, (h + 1) * 512)

            # ---- Phase B: o-major merged k-sweeps (64 MMs) at pure PE rate.
            # The last o-tile runs four sequential 256-column quarter-sweeps
            # so only one small eviction + DMA trails the final matmul.
            for o in range(A_TILES, NB):
                if o + 2 < NB:
                    load_wt(o + 2, 0, KO)
                wto = wt_tile(o)
                if o == NB - 1:
                    for q in range(4):
                        ps = psum_pool.tile(
                            [P, 256], mybir.dt.float32, name=f"ps_{o}_q{q}", tag="ps"
                        )
                        for ko in range(KO):
                            nc.tensor.matmul(
                                ps[:],
                                lhsT=wto[:, ko, :],
                                rhs=xt_sb[:, ko, q * 256 : (q + 1) * 256],
                                start=(ko == 0),
                                stop=(ko == KO - 1),
                            )
                        evict(o, ps, q * 256, (q + 1) * 256)
                    continue
                ps = {
                    h: psum_pool.tile(
                        [P, 512], mybir.dt.float32, name=f"ps_{o}_{h}", tag="ps"
                    )
                    for h in range(NH)
                }
                for ko in range(KO):
                    for h in range(NH):
                        nc.tensor.matmul(
                            ps[h][:],
                            lhsT=wto[:, ko, :],
                            rhs=xt_sb[:, ko, h * 512 : (h + 1) * 512],
                            start=(ko == 0),
                            stop=(ko == KO - 1),
                        )
                for h in range(NH):
                    evict(o, ps[h], h * 512, (h + 1) * 512)

    nc.compile()
    _CACHE[key] = nc
    return nc


def _densify_a(values, row_ids, col_ids, in_f=IN_F, out_f=OUT_F):
    """A[i, o] = sum of values[k] over k with col_ids[k]==i, row_ids[k]==o."""
    idx = col_ids.astype(np.int64) * out_f + row_ids.astype(np.int64)
    a = np.bincount(idx, weights=values.astype(np.float64), minlength=in_f * out_f)
    return a.astype(np.float32).reshape(in_f, out_f)


def kernel(x, values, row_ids, col_ids, bias):
    from concourse import bass_utils

    if os.environ.get("BASS_TRACE"):
        _ensure_ntff_hook()
        _patch_upload()

    nc = build_program()

    x = np.asarray(x, dtype=np.float32)
    values = np.asarray(values, dtype=np.float32)
    row_ids = np.asarray(row_ids)
    col_ids = np.asarray(col_ids)
    bias = np.asarray(bias, dtype=np.float32)

    KO = IN_F // P
    NB = OUT_F // P
    tpc = TOKENS // N_CORES

    a = _densify_a(values, row_ids, col_ids)  # [in_f, out_f] f32
    # wt2[nb, p, ko, o] = A[ko*128+p, nb*128+o]
    wt2 = np.ascontiguousarray(
        a.reshape(KO, P, NB, P).transpose(2, 1, 0, 3).reshape(OUT_F, IN_F)
    ).astype(ml_dtypes.bfloat16)
    bias2 = np.ascontiguousarray(bias.reshape(NB, P).T).astype(np.float32)

    in_maps = []
    for c in range(N_CORES):
        xT = x[c * tpc : (c + 1) * tpc, :].T  # [in_f, tpc]
        xt2 = np.ascontiguousarray(
            xT.reshape(KO, P, tpc).transpose(1, 0, 2).reshape(P, KO * tpc)
        ).astype(ml_dtypes.bfloat16)
        in_maps.append({"xt": xt2, "wt": wt2, "biasr": bias2})

    res = bass_utils.run_bass_kernel_spmd(nc, in_maps, core_ids=list(range(N_CORES)))
    global last_results
    last_results = res
    return np.ascontiguousarray(
        np.concatenate(
            [res.results[c]["out"].T.astype(np.float32) for c in range(N_CORES)],
            axis=0,
        )
    )


last_results = None


# revision 38
# speedup vs baseline: 1.0030x; 1.0030x over previous
"""CSR Linear kernel for TRN2: out = x @ W^T + bias, W from COO nonzeros.

Verified fallback config (three runs: 462667/462772/463258 ns, rel err
2.87e-3): pure bf16, weight-stationary, phased DMA, PE pre-warm, bf16 out.
"""

import os
import sys
import types

import ml_dtypes
import numpy as np

TOKENS = 8192
IN_F = 4096
OUT_F = 4096
N_CORES = 8
P = 128

_CACHE = {}


def _ensure_ntff_hook():
    try:
        import antenv.axon_hooks  # noqa: F401

        return
    except ImportError:
        pass
    try:
        import antenv
        from trn_agent_boot.trn_boot import _ntff_profile_via_ctypes

        hooks = types.ModuleType("antenv.axon_hooks")
        hooks._hook = _ntff_profile_via_ctypes("/opt/axon/libaxon_pjrt.so")
        hooks.set_axon_ntff_profile_hook = lambda h: setattr(hooks, "_hook", h)
        hooks.get_axon_ntff_profile_hook = lambda: hooks._hook
        sys.modules["antenv.axon_hooks"] = hooks
        antenv.axon_hooks = hooks
    except Exception:
        pass


def _patch_upload():
    from concourse import bass_utils

    orig = bass_utils.upload_artifacts
    if getattr(orig, "_kernel_patched", False):
        return

    def _safe_upload(tmpdir):
        try:
            return orig(tmpdir)
        except Exception:
            return tmpdir

    _safe_upload._kernel_patched = True
    bass_utils.upload_artifacts = _safe_upload


def build_program(tok_per_core=TOKENS // N_CORES, in_f=IN_F, out_f=OUT_F):
    key = (tok_per_core, in_f, out_f)
    if key in _CACHE:
        return _CACHE[key]

    import concourse.bacc as bacc
    import concourse.mybir as mybir
    import concourse.tile as tile

    KO = in_f // P
    NB = out_f // P
    NH = tok_per_core // 512
    A_TILES = 4

    nc = bacc.Bacc("TRN2", target_bir_lowering=False, debug=False)
    nc.m.queues = [q for q in nc.m.queues if q.name != "qScalarDynamicHW"]

    xt = nc.dram_tensor("xt", [P, KO * tok_per_core], mybir.dt.bfloat16, kind="ExternalInput")
    wt = nc.dram_tensor("wt", [out_f, in_f], mybir.dt.bfloat16, kind="ExternalInput")
    biasr = nc.dram_tensor("biasr", [P, NB], mybir.dt.float32, kind="ExternalInput")
    out = nc.dram_tensor("out", [out_f, tok_per_core], mybir.dt.bfloat16, kind="ExternalOutput")

    xt_ap = xt.ap().rearrange("p (ko t) -> p ko t", ko=KO)
    wt_ap = wt.ap().rearrange("(nb p) (ko o) -> p nb ko o", p=P, o=P)
    out_ap = out.ap().rearrange("(nb p) t -> p nb t", p=P)

    with tile.TileContext(nc) as tc:
        with (
            tc.tile_pool(name="xt_pool", bufs=1) as xt_pool,
            tc.tile_pool(name="warm_pool", bufs=1) as warm_pool,
            tc.tile_pool(name="bias_pool", bufs=1) as bias_pool,
            tc.tile_pool(name="wt_pool", bufs=5) as wt_pool,
            tc.tile_pool(name="out_pool", bufs=4) as out_pool,
            tc.tile_pool(name="psum", bufs=8, space="PSUM") as psum_pool,
        ):
            xt_sb = xt_pool.tile([P, KO, tok_per_core], mybir.dt.bfloat16)
            bias_sb = bias_pool.tile([P, NB], mybir.dt.float32)

            wt_tiles = {}

            def wt_tile(o):
                if o not in wt_tiles:
                    wt_tiles[o] = wt_pool.tile(
                        [P, KO, P], mybir.dt.bfloat16, name=f"wt_{o}", tag="wt"
                    )
                return wt_tiles[o]

            def load_wt(o, kb, kbe):
                nc.sync.dma_start(wt_tile(o)[:, kb:kbe, :], wt_ap[:, o, kb:kbe, :])

            def load_xt(kb, kbe):
                nc.sync.dma_start(xt_sb[:, kb:kbe, :], xt_ap[:, kb:kbe, :])

            def evict(o, ps, cb, cbe):
                ot = out_pool.tile(
                    [P, cbe - cb], mybir.dt.bfloat16, name=f"ot_{o}_{cb}", tag="ot"
                )
                nc.vector.tensor_scalar_add(ot[:], ps[:], bias_sb[:, o : o + 1])
                nc.sync.dma_start(out_ap[:, o, cb:cbe], ot[:])

            warm_x = warm_pool.tile([P, 512], mybir.dt.bfloat16, name="warm_x")
            nc.vector.memset(warm_x[:], 0.0)
            warm_ps = psum_pool.tile([P, 512], mybir.dt.float32, name="warm_ps", tag="ps")
            for _ in range(6):
                nc.tensor.matmul(
                    warm_ps[:], lhsT=warm_x[:, 0:P], rhs=warm_x[:], start=True, stop=True
                )

            chunks = [(0, 1), (1, 4), (4, 8), (8, 16), (16, 24), (24, 32)]
            xt_pieces = [(0, 1), (1, 2), (2, 4), (4, 6), (6, 8)] + [
                (b, b + 2) for b in range(8, KO, 2)
            ]
            ps_a = {
                (o, h): psum_pool.tile([P, 512], mybir.dt.float32, name=f"psA_{o}_{h}", tag="ps")
                for o in range(A_TILES)
                for h in range(NH)
            }
            for ci, (kb, kbe) in enumerate(chunks):
                for pb, pbe in xt_pieces:
                    if pb >= kb and pbe <= kbe:
                        load_xt(pb, pbe)
                for o in range(A_TILES):
                    load_wt(o, kb, kbe)
                if ci == 2:
                    nc.sync.dma_start(bias_sb[:], biasr.ap())
                if ci == len(chunks) - 2:
                    load_wt(A_TILES, 0, KO)
                if ci == len(chunks) - 1:
                    load_wt(A_TILES + 1, 0, KO)
            for kb, kbe in chunks:
                for o in range(A_TILES):
                    wto = wt_tile(o)
                    for ko in range(kb, kbe):
                        for h in range(NH):
                            nc.tensor.matmul(
                                ps_a[(o, h)][:],
                                lhsT=wto[:, ko, :],
                                rhs=xt_sb[:, ko, h * 512 : (h + 1) * 512],
                                start=(ko == 0),
                                stop=(ko == KO - 1),
                            )
            for o in range(A_TILES):
                for h in range(NH):
                    evict(o, ps_a[(o, h)], h * 512, (h + 1) * 512)

            for o in range(A_TILES, NB):
                if o + 2 < NB:
                    load_wt(o + 2, 0, KO)
                wto = wt_tile(o)
                if o == NB - 1:
                    for q in range(4):
                        ps = psum_pool.tile(
                            [P, 256], mybir.dt.float32, name=f"ps_{o}_q{q}", tag="ps"
                        )
                        for ko in range(KO):
                            nc.tensor.matmul(
                                ps[:],
                                lhsT=wto[:, ko, :],
                                rhs=xt_sb[:, ko, q * 256 : (q + 1) * 256],
                                start=(ko == 0),
                                stop=(ko == KO - 1),
                            )
                        evict(o, ps, q * 256, (q + 1) * 256)
                    continue
                ps = {
                    h: psum_pool.tile(
                        [P, 512], mybir.dt.float32, name=f"ps_{o}_{h}", tag="ps"
                    )
                    for h in range(NH)
                }
                for ko in range(KO):
                    for h in range(NH):
                        nc.tensor.matmul(
                            ps[h][:],
                            lhsT=wto[:, ko, :],
                            rhs=xt_sb[:, ko, h * 512 : (h + 1) * 512],
                            start=(ko == 0),
                            stop=(ko == KO - 1),
                        )
                for h in range(NH):
                    evict(o, ps[h], h * 512, (h + 1) * 512)

    nc.compile()
    _CACHE[key] = nc
    return nc


def _densify_a(values, row_ids, col_ids, in_f=IN_F, out_f=OUT_F):
    idx = col_ids.astype(np.int64) * out_f + row_ids.astype(np.int64)
    a = np.bincount(idx, weights=values.astype(np.float64), minlength=in_f * out_f)
    return a.astype(np.float32).reshape(in_f, out_f)


def kernel(x, values, row_ids, col_ids, bias):
    from concourse import bass_utils

    if os.environ.get("BASS_TRACE"):
        _ensure_ntff_hook()
        _patch_upload()

    nc = build_program()

    x = np.asarray(x, dtype=np.float32)
    values = np.asarray(values, dtype=np.float32)
    row_ids = np.asarray(row_ids)
    col_ids = np.asarray(col_ids)
    bias = np.asarray(bias, dtype=np.float32)

    KO = IN_F // P
    NB = OUT_F // P
    tpc = TOKENS // N_CORES

    a = _densify_a(values, row_ids, col_ids)
    wt2 = np.ascontiguousarray(
        a.reshape(KO, P, NB, P).transpose(2, 1, 0, 3).reshape(OUT_F, IN_F)
    ).astype(ml_dtypes.bfloat16)
    bias2 = np.ascontiguousarray(bias.reshape(NB, P).T).astype(np.float32)

    in_maps = []
    for c in range(N_CORES):
        xT = x[c * tpc : (c + 1) * tpc, :].T
        xt2 = np.ascontiguousarray(
            xT.reshape(KO, P, tpc).transpose(1, 0, 2).reshape(P, KO * tpc)
        ).astype(ml_dtypes.bfloat16)
        in_maps.append({"xt": xt2, "wt": wt2, "biasr": bias2})

    res = bass_utils.run_bass_kernel_spmd(nc, in_maps, core_ids=list(range(N_CORES)))
    global last_results
    last_results = res
    return np.ascontiguousarray(
        np.concatenate(
            [res.results[c]["out"].T.astype(np.float32) for c in range(N_CORES)],
            axis=0,
        )
    )


last_results = None


# revision 44
# speedup vs baseline: 1.0690x; 1.0657x over previous
"""CSR Linear kernel for TRN2: out = x @ W^T + bias, W from COO nonzeros.

Strategy: data-parallel over tokens across 8 NeuronCores. Host densifies the
sparse weight into A[in, out] (duplicate coords summed); each core computes
its 1024-token shard as out^T with A-tiles stationary on the PE and x^T
streaming: psum[128 outf, 512 tok] += A_tile[128 k, 128 outf].T @ xT[...].

Mixed precision: 26 of 32 k-tiles run in bf16 (1 col/cycle), the last 6
k-tiles run in fp8-e4m3 DoubleRow (2 MACs/cell/cycle, 3 matmuls of 256-wide
contraction) — host-simulated end-to-end rel err 1.64e-2 vs the 2e-2 gate,
for a ~13% cut in PE streaming time. Out-features sit on PSUM partitions so
the bias is a per-partition tensor_scalar on the bf16 eviction. Phase A runs
the first 4 out-tiles k-outer so DMA demand tracks the HBM ramp from the
first matmul; phase B runs o-major k-sweeps at pure PE rate with per-sweep
evictions. A short dummy-matmul block pre-warms the PE clock (HAM) during
the DMA startup hole.
"""

import os
import sys
import types

import ml_dtypes
import numpy as np

TOKENS = 8192
IN_F = 4096
OUT_F = 4096
N_CORES = 8
P = 128
KO_BF = 26  # bf16 k-tiles
KF8 = 6  # fp8 k-tiles (3 DoubleRow pairs)
SPLIT = KO_BF * P  # in-feature split point

_CACHE = {}


def _ensure_ntff_hook():
    try:
        import antenv.axon_hooks  # noqa: F401

        return
    except ImportError:
        pass
    try:
        import antenv
        from trn_agent_boot.trn_boot import _ntff_profile_via_ctypes

        hooks = types.ModuleType("antenv.axon_hooks")
        hooks._hook = _ntff_profile_via_ctypes("/opt/axon/libaxon_pjrt.so")
        hooks.set_axon_ntff_profile_hook = lambda h: setattr(hooks, "_hook", h)
        hooks.get_axon_ntff_profile_hook = lambda: hooks._hook
        sys.modules["antenv.axon_hooks"] = hooks
        antenv.axon_hooks = hooks
    except Exception:
        pass


def _patch_upload():
    from concourse import bass_utils

    orig = bass_utils.upload_artifacts
    if getattr(orig, "_kernel_patched", False):
        return

    def _safe_upload(tmpdir):
        try:
            return orig(tmpdir)
        except Exception:
            return tmpdir

    _safe_upload._kernel_patched = True
    bass_utils.upload_artifacts = _safe_upload


def build_program(tok_per_core=TOKENS // N_CORES, in_f=IN_F, out_f=OUT_F):
    key = (tok_per_core, in_f, out_f)
    if key in _CACHE:
        return _CACHE[key]

    import concourse.bacc as bacc
    import concourse.mybir as mybir
    import concourse.tile as tile

    NB = out_f // P  # 32 out-feature tiles
    NH = tok_per_core // 512  # 2 token halves (psum bank = 512 f32)
    NDR = KF8 // 2  # DoubleRow pair count
    A_TILES = 4  # phase-A out-tiles (k-outer), 4*NH = 8 psum banks

    nc = bacc.Bacc("TRN2", target_bir_lowering=False, debug=False)
    nc.m.queues = [q for q in nc.m.queues if q.name != "qScalarDynamicHW"]

    xt = nc.dram_tensor("xt", [P, KO_BF * tok_per_core], mybir.dt.bfloat16, kind="ExternalInput")
    x8 = nc.dram_tensor("x8", [P, KF8 * tok_per_core], mybir.dt.float8e4, kind="ExternalInput")
    wt = nc.dram_tensor("wt", [out_f, KO_BF * P], mybir.dt.bfloat16, kind="ExternalInput")
    w8 = nc.dram_tensor("w8", [out_f, KF8 * P], mybir.dt.float8e4, kind="ExternalInput")
    biasr = nc.dram_tensor("biasr", [P, NB], mybir.dt.float32, kind="ExternalInput")
    # outT[nb*128+p, t] = out[t, nb*128+p]; bf16, host upcasts.
    out = nc.dram_tensor("out", [out_f, tok_per_core], mybir.dt.bfloat16, kind="ExternalOutput")

    xt_ap = xt.ap().rearrange("p (ko t) -> p ko t", ko=KO_BF)
    x8_ap = x8.ap().rearrange("p (ko t) -> p ko t", ko=KF8)
    wt_ap = wt.ap().rearrange("(nb p) (ko o) -> p nb ko o", p=P, o=P)
    w8_ap = w8.ap().rearrange("(nb p) (ko o) -> p nb ko o", p=P, o=P)
    out_ap = out.ap().rearrange("(nb p) t -> p nb t", p=P)

    with tile.TileContext(nc) as tc:
        with (
            tc.tile_pool(name="xt_pool", bufs=1) as xt_pool,
            tc.tile_pool(name="x8_pool", bufs=1) as x8_pool,
            tc.tile_pool(name="warm_pool", bufs=1) as warm_pool,
            tc.tile_pool(name="bias_pool", bufs=1) as bias_pool,
            tc.tile_pool(name="wt_pool", bufs=5) as wt_pool,
            tc.tile_pool(name="out_pool", bufs=4) as out_pool,
            tc.tile_pool(name="psum", bufs=8, space="PSUM") as psum_pool,
        ):
            xt_sb = xt_pool.tile([P, KO_BF, tok_per_core], mybir.dt.bfloat16)
            x8_sb = x8_pool.tile([P, KF8, tok_per_core], mybir.dt.float8e4)
            bias_sb = bias_pool.tile([P, NB], mybir.dt.float32)

            wt_tiles = {}
            w8_sb = x8_pool.tile([P, NB, KF8, P], mybir.dt.float8e4, name="w8_sb")

            def wt_tile(o):
                if o not in wt_tiles:
                    wt_tiles[o] = wt_pool.tile(
                        [P, KO_BF, P], mybir.dt.bfloat16, name=f"wt_{o}", tag="wt"
                    )
                return wt_tiles[o]

            def load_wt(o, kb, kbe):
                nc.sync.dma_start(wt_tile(o)[:, kb:kbe, :], wt_ap[:, o, kb:kbe, :])

            def load_xt(kb, kbe):
                nc.sync.dma_start(xt_sb[:, kb:kbe, :], xt_ap[:, kb:kbe, :])

            def evict(o, ps, cb, cbe):
                ot = out_pool.tile(
                    [P, cbe - cb], mybir.dt.bfloat16, name=f"ot_{o}_{cb}", tag="ot"
                )
                nc.vector.tensor_scalar_add(ot[:], ps[:], bias_sb[:, o : o + 1])
                nc.sync.dma_start(out_ap[:, o, cb:cbe], ot[:])

            def mm_bf(ps, o, ko, cb, cbe, start):
                nc.tensor.matmul(
                    ps[:],
                    lhsT=wt_tile(o)[:, ko, :],
                    rhs=xt_sb[:, ko, cb:cbe],
                    start=start,
                    stop=(ko == KO_BF - 1),
                )

            def mm_dr(ps, o, b, cb, cbe):
                # fp8 DoubleRow matmuls form their own pure-dtype PSUM group
                nc.tensor.matmul(
                    ps[:],
                    lhsT=w8_sb[:, o, 2 * b : 2 * b + 2, :],
                    rhs=x8_sb[:, 2 * b : 2 * b + 2, cb:cbe],
                    start=(b == 0),
                    stop=(b == NDR - 1),
                    perf_mode=mybir.MatmulPerfMode.DoubleRow,
                )

            def evict2(o, stage, ps_f8, cb, cbe):
                # stage holds (bf16-group psum + bias) in f32 SBUF; add the
                # fp8-group psum and emit bf16 output.
                ot = out_pool.tile(
                    [P, cbe - cb], mybir.dt.bfloat16, name=f"ot_{o}_{cb}", tag="ot"
                )
                nc.vector.tensor_add(out=ot[:], in0=stage[:], in1=ps_f8[:])
                nc.sync.dma_start(out_ap[:, o, cb:cbe], ot[:])

            def stage_bf(o, ps_bf, cb, cbe):
                st = out_pool.tile(
                    [P, cbe - cb], mybir.dt.float32, name=f"st_{o}_{cb}", tag="st"
                )
                nc.vector.tensor_scalar_add(st[:], ps_bf[:], bias_sb[:, o : o + 1])
                return st

            # ---- PE pre-warm: dummy matmuls on scratch during the DMA
            # startup hole so HAM un-throttles before the first real matmul.
            warm_x = warm_pool.tile([P, 512], mybir.dt.bfloat16, name="warm_x")
            nc.vector.memset(warm_x[:], 0.0)
            warm_ps = psum_pool.tile([P, 512], mybir.dt.float32, name="warm_ps", tag="ps")
            for _ in range(6):
                nc.tensor.matmul(
                    warm_ps[:], lhsT=warm_x[:, 0:P], rhs=warm_x[:], start=True, stop=True
                )

            # ---- Phase A: out-tiles 0..3, k-outer so DMA demand is smooth ----
            chunks = [(0, 1), (1, 4), (4, 8), (8, 16), (16, 20), (20, KO_BF)]
            xt_pieces = [(0, 1), (1, 2), (2, 4), (4, 6), (6, 8)] + [
                (b, min(b + 2, KO_BF)) for b in range(8, KO_BF, 2)
            ]
            ps_a = {
                (o, h): psum_pool.tile([P, 512], mybir.dt.float32, name=f"psA_{o}_{h}", tag="ps")
                for o in range(A_TILES)
                for h in range(NH)
            }
            for ci, (kb, kbe) in enumerate(chunks):
                for pb, pbe in xt_pieces:
                    if pb >= kb and pbe <= kbe:
                        load_xt(pb, pbe)
                for o in range(A_TILES):
                    load_wt(o, kb, kbe)
                if ci == 2:
                    nc.sync.dma_start(bias_sb[:], biasr.ap())
                if ci == 1:
                    nc.sync.dma_start(w8_sb[:], w8_ap[:])
                if ci == len(chunks) - 2:
                    nc.sync.dma_start(x8_sb[:, 0:3, :], x8_ap[:, 0:3, :])
                    nc.sync.dma_start(x8_sb[:, 3:KF8, :], x8_ap[:, 3:KF8, :])
                    load_wt(A_TILES, 0, KO_BF)  # phase-B prefetch into spare bufs
                if ci == len(chunks) - 1:
                    load_wt(A_TILES + 1, 0, KO_BF)
            for kb, kbe in chunks:
                for o in range(A_TILES):
                    for ko in range(kb, kbe):
                        for h in range(NH):
                            mm_bf(ps_a[(o, h)], o, ko, h * 512, (h + 1) * 512, ko == 0)
            for o in range(A_TILES):
                for h in range(NH):
                    st = stage_bf(o, ps_a[(o, h)], h * 512, (h + 1) * 512)
                    ps8 = psum_pool.tile(
                        [P, 512], mybir.dt.float32, name=f"ps8A_{o}_{h}", tag="ps"
                    )
                    for b in range(NDR):
                        mm_dr(ps8, o, b, h * 512, (h + 1) * 512)
                    evict2(o, st, ps8, h * 512, (h + 1) * 512)

            # ---- Phase B: o-major merged k-sweeps at pure PE rate.
            # The last o-tile runs four sequential 256-column quarter-sweeps
            # so only one small eviction + DMA trails the final matmul.
            for o in range(A_TILES, NB):
                if o + 2 < NB:
                    load_wt(o + 2, 0, KO_BF)
                if o == NB - 1:
                    for q in range(4):
                        ps = psum_pool.tile(
                            [P, 256], mybir.dt.float32, name=f"ps_{o}_q{q}", tag="ps"
                        )
                        for ko in range(KO_BF):
                            mm_bf(ps, o, ko, q * 256, (q + 1) * 256, ko == 0)
                        stq = stage_bf(o, ps, q * 256, (q + 1) * 256)
                        ps8 = psum_pool.tile(
                            [P, 256], mybir.dt.float32, name=f"ps8_{o}_q{q}", tag="ps"
                        )
                        for b in range(NDR):
                            mm_dr(ps8, o, b, q * 256, (q + 1) * 256)
                        evict2(o, stq, ps8, q * 256, (q + 1) * 256)
                    continue
                ps = {
                    h: psum_pool.tile(
                        [P, 512], mybir.dt.float32, name=f"ps_{o}_{h}", tag="ps"
                    )
                    for h in range(NH)
                }
                for ko in range(KO_BF):
                    for h in range(NH):
                        mm_bf(ps[h], o, ko, h * 512, (h + 1) * 512, ko == 0)
                ps8 = {
                    h: psum_pool.tile(
                        [P, 512], mybir.dt.float32, name=f"ps8_{o}_{h}", tag="ps"
                    )
                    for h in range(NH)
                }
                for b in range(NDR):
                    for h in range(NH):
                        mm_dr(ps8[h], o, b, h * 512, (h + 1) * 512)
                for h in range(NH):
                    st = stage_bf(o, ps[h], h * 512, (h + 1) * 512)
                    evict2(o, st, ps8[h], h * 512, (h + 1) * 512)

    nc.compile()
    _CACHE[key] = nc
    return nc


def _densify_a(values, row_ids, col_ids, in_f=IN_F, out_f=OUT_F):
    """A[i, o] = sum of values[k] over k with col_ids[k]==i, row_ids[k]==o."""
    idx = col_ids.astype(np.int64) * out_f + row_ids.astype(np.int64)
    a = np.bincount(idx, weights=values.astype(np.float64), minlength=in_f * out_f)
    return a.astype(np.float32).reshape(in_f, out_f)


def kernel(x, values, row_ids, col_ids, bias):
    from concourse import bass_utils

    if os.environ.get("BASS_TRACE"):
        _ensure_ntff_hook()
        _patch_upload()

    nc = build_program()

    x = np.asarray(x, dtype=np.float32)
    values = np.asarray(values, dtype=np.float32)
    row_ids = np.asarray(row_ids)
    col_ids = np.asarray(col_ids)
    bias = np.asarray(bias, dtype=np.float32)

    NB = OUT_F // P
    tpc = TOKENS // N_CORES
    bf16 = ml_dtypes.bfloat16
    e4m3 = ml_dtypes.float8_e4m3

    a = _densify_a(values, row_ids, col_ids)  # [in_f, out_f] f32
    wt2 = np.ascontiguousarray(
        a[:SPLIT].reshape(KO_BF, P, NB, P).transpose(2, 1, 0, 3).reshape(OUT_F, KO_BF * P)
    ).astype(bf16)
    w82 = np.ascontiguousarray(
        a[SPLIT:].reshape(KF8, P, NB, P).transpose(2, 1, 0, 3).reshape(OUT_F, KF8 * P)
    ).astype(e4m3)
    bias2 = np.ascontiguousarray(bias.reshape(NB, P).T).astype(np.float32)

    in_maps = []
    for c in range(N_CORES):
        xT = x[c * tpc : (c + 1) * tpc, :].T  # [in_f, tpc]
        xt2 = np.ascontiguousarray(
            xT[:SPLIT].reshape(KO_BF, P, tpc).transpose(1, 0, 2).reshape(P, KO_BF * tpc)
        ).astype(bf16)
        x82 = np.ascontiguousarray(
            xT[SPLIT:].reshape(KF8, P, tpc).transpose(1, 0, 2).reshape(P, KF8 * tpc)
        ).astype(e4m3)
        in_maps.append({"xt": xt2, "x8": x82, "wt": wt2, "w8": w82, "biasr": bias2})

    res = bass_utils.run_bass_kernel_spmd(nc, in_maps, core_ids=list(range(N_CORES)))
    global last_results
    last_results = res
    return np.ascontiguousarray(
        np.concatenate(
            [res.results[c]["out"].T.astype(np.float32) for c in range(N_CORES)],
            axis=0,
        )
    )


last_results = None


# revision 45
# speedup vs baseline: 1.0860x; 1.0160x over previous
"""CSR Linear kernel for TRN2: out = x @ W^T + bias, W from COO nonzeros.

Strategy: data-parallel over tokens across 8 NeuronCores. Host densifies the
sparse weight into A[in, out] (duplicate coords summed); each core computes
its 1024-token shard as out^T with A-tiles stationary on the PE and x^T
streaming: psum[128 outf, 512 tok] += A_tile[128 k, 128 outf].T @ xT[...].

Mixed precision: 26 of 32 k-tiles run in bf16 (1 col/cycle), the last 6
k-tiles run in fp8-e4m3 DoubleRow (2 MACs/cell/cycle, 3 matmuls of 256-wide
contraction) — host-simulated end-to-end rel err 1.64e-2 vs the 2e-2 gate,
for a ~13% cut in PE streaming time. Out-features sit on PSUM partitions so
the bias is a per-partition tensor_scalar on the bf16 eviction. Phase A runs
the first 4 out-tiles k-outer so DMA demand tracks the HBM ramp from the
first matmul; phase B runs o-major k-sweeps at pure PE rate with per-sweep
evictions. A short dummy-matmul block pre-warms the PE clock (HAM) during
the DMA startup hole.
"""

import os
import sys
import types

import ml_dtypes
import numpy as np

TOKENS = 8192
IN_F = 4096
OUT_F = 4096
N_CORES = 8
P = 128
KO_BF = 26  # bf16 k-tiles
KF8 = 6  # fp8 k-tiles (3 DoubleRow pairs)
SPLIT = KO_BF * P  # in-feature split point

_CACHE = {}


def _ensure_ntff_hook():
    try:
        import antenv.axon_hooks  # noqa: F401

        return
    except ImportError:
        pass
    try:
        import antenv
        from trn_agent_boot.trn_boot import _ntff_profile_via_ctypes

        hooks = types.ModuleType("antenv.axon_hooks")
        hooks._hook = _ntff_profile_via_ctypes("/opt/axon/libaxon_pjrt.so")
        hooks.set_axon_ntff_profile_hook = lambda h: setattr(hooks, "_hook", h)
        hooks.get_axon_ntff_profile_hook = lambda: hooks._hook
        sys.modules["antenv.axon_hooks"] = hooks
        antenv.axon_hooks = hooks
    except Exception:
        pass


def _patch_upload():
    from concourse import bass_utils

    orig = bass_utils.upload_artifacts
    if getattr(orig, "_kernel_patched", False):
        return

    def _safe_upload(tmpdir):
        try:
            return orig(tmpdir)
        except Exception:
            return tmpdir

    _safe_upload._kernel_patched = True
    bass_utils.upload_artifacts = _safe_upload


def build_program(tok_per_core=TOKENS // N_CORES, in_f=IN_F, out_f=OUT_F):
    key = (tok_per_core, in_f, out_f)
    if key in _CACHE:
        return _CACHE[key]

    import concourse.bacc as bacc
    import concourse.mybir as mybir
    import concourse.tile as tile

    NB = out_f // P  # 32 out-feature tiles
    NH = tok_per_core // 512  # 2 token halves (psum bank = 512 f32)
    NDR = KF8 // 2  # DoubleRow pair count
    A_TILES = 4  # phase-A out-tiles (k-outer), 4*NH = 8 psum banks

    nc = bacc.Bacc("TRN2", target_bir_lowering=False, debug=False)
    nc.m.queues = [q for q in nc.m.queues if q.name != "qScalarDynamicHW"]

    xt = nc.dram_tensor("xt", [P, KO_BF * tok_per_core], mybir.dt.bfloat16, kind="ExternalInput")
    x8 = nc.dram_tensor("x8", [P, KF8 * tok_per_core], mybir.dt.float8e4, kind="ExternalInput")
    wt = nc.dram_tensor("wt", [out_f, KO_BF * P], mybir.dt.bfloat16, kind="ExternalInput")
    w8 = nc.dram_tensor("w8", [out_f, KF8 * P], mybir.dt.float8e4, kind="ExternalInput")
    biasr = nc.dram_tensor("biasr", [P, NB], mybir.dt.float32, kind="ExternalInput")
    # outT[nb*128+p, t] = out[t, nb*128+p]; bf16, host upcasts.
    out = nc.dram_tensor("out", [out_f, tok_per_core], mybir.dt.bfloat16, kind="ExternalOutput")

    xt_ap = xt.ap().rearrange("p (ko t) -> p ko t", ko=KO_BF)
    x8_ap = x8.ap().rearrange("p (ko t) -> p ko t", ko=KF8)
    wt_ap = wt.ap().rearrange("(nb p) (ko o) -> p nb ko o", p=P, o=P)
    w8_ap = w8.ap().rearrange("(nb p) (ko o) -> p nb ko o", p=P, o=P)
    out_ap = out.ap().rearrange("(nb p) t -> p nb t", p=P)

    with tile.TileContext(nc) as tc:
        with (
            tc.tile_pool(name="xt_pool", bufs=1) as xt_pool,
            tc.tile_pool(name="x8_pool", bufs=1) as x8_pool,
            tc.tile_pool(name="warm_pool", bufs=1) as warm_pool,
            tc.tile_pool(name="bias_pool", bufs=1) as bias_pool,
            tc.tile_pool(name="wt_pool", bufs=5) as wt_pool,
            tc.tile_pool(name="out_pool", bufs=4) as out_pool,
            tc.tile_pool(name="psum", bufs=8, space="PSUM") as psum_pool,
        ):
            xt_sb = xt_pool.tile([P, KO_BF, tok_per_core], mybir.dt.bfloat16)
            x8_sb = x8_pool.tile([P, KF8, tok_per_core], mybir.dt.float8e4)
            bias_sb = bias_pool.tile([P, NB], mybir.dt.float32)

            wt_tiles = {}
            w8_sb = x8_pool.tile([P, NB, KF8, P], mybir.dt.float8e4, name="w8_sb")

            def wt_tile(o):
                if o not in wt_tiles:
                    wt_tiles[o] = wt_pool.tile(
                        [P, KO_BF, P], mybir.dt.bfloat16, name=f"wt_{o}", tag="wt"
                    )
                return wt_tiles[o]

            def load_wt(o, kb, kbe):
                nc.sync.dma_start(wt_tile(o)[:, kb:kbe, :], wt_ap[:, o, kb:kbe, :])

            def load_xt(kb, kbe):
                nc.sync.dma_start(xt_sb[:, kb:kbe, :], xt_ap[:, kb:kbe, :])

            def evict(o, ps, cb, cbe):
                ot = out_pool.tile(
                    [P, cbe - cb], mybir.dt.bfloat16, name=f"ot_{o}_{cb}", tag="ot"
                )
                nc.vector.tensor_scalar_add(ot[:], ps[:], bias_sb[:, o : o + 1])
                nc.sync.dma_start(out_ap[:, o, cb:cbe], ot[:])

            def mm_bf(ps, o, ko, cb, cbe, start):
                nc.tensor.matmul(
                    ps[:],
                    lhsT=wt_tile(o)[:, ko, :],
                    rhs=xt_sb[:, ko, cb:cbe],
                    start=start,
                    stop=(ko == KO_BF - 1),
                )

            def mm_dr(ps, o, b, cb, cbe):
                # fp8 DoubleRow matmuls form their own pure-dtype PSUM group
                nc.tensor.matmul(
                    ps[:],
                    lhsT=w8_sb[:, o, 2 * b : 2 * b + 2, :],
                    rhs=x8_sb[:, 2 * b : 2 * b + 2, cb:cbe],
                    start=(b == 0),
                    stop=(b == NDR - 1),
                    perf_mode=mybir.MatmulPerfMode.DoubleRow,
                )

            def evict2(o, stage, ps_f8, cb, cbe):
                # stage holds (bf16-group psum + bias) in f32 SBUF; add the
                # fp8-group psum and emit bf16 output.
                ot = out_pool.tile(
                    [P, cbe - cb], mybir.dt.bfloat16, name=f"ot_{o}_{cb}", tag="ot"
                )
                nc.vector.tensor_add(out=ot[:], in0=stage[:], in1=ps_f8[:])
                nc.sync.dma_start(out_ap[:, o, cb:cbe], ot[:])

            def stage_bf(o, ps_bf, cb, cbe):
                st = out_pool.tile(
                    [P, cbe - cb], mybir.dt.float32, name=f"st_{o}_{cb}", tag="st"
                )
                nc.vector.tensor_scalar_add(st[:], ps_bf[:], bias_sb[:, o : o + 1])
                return st

            # ---- PE pre-warm: dummy matmuls on scratch during the DMA
            # startup hole so HAM un-throttles before the first real matmul.
            warm_x = warm_pool.tile([P, 512], mybir.dt.bfloat16, name="warm_x")
            nc.vector.memset(warm_x[:], 0.0)
            warm_ps = psum_pool.tile([P, 512], mybir.dt.float32, name="warm_ps", tag="ps")
            for _ in range(6):
                nc.tensor.matmul(
                    warm_ps[:], lhsT=warm_x[:, 0:P], rhs=warm_x[:], start=True, stop=True
                )

            # ---- Phase A: out-tiles 0..3, k-outer so DMA demand is smooth ----
            chunks = [(0, 1), (1, 4), (4, 8), (8, 16), (16, 20), (20, KO_BF)]
            xt_pieces = [(0, 1), (1, 2), (2, 4), (4, 6), (6, 8)] + [
                (b, min(b + 2, KO_BF)) for b in range(8, KO_BF, 2)
            ]
            ps_a = {
                (o, h): psum_pool.tile([P, 512], mybir.dt.float32, name=f"psA_{o}_{h}", tag="ps")
                for o in range(A_TILES)
                for h in range(NH)
            }
            for ci, (kb, kbe) in enumerate(chunks):
                for pb, pbe in xt_pieces:
                    if pb >= kb and pbe <= kbe:
                        load_xt(pb, pbe)
                for o in range(A_TILES):
                    load_wt(o, kb, kbe)
                if ci == 2:
                    nc.sync.dma_start(bias_sb[:], biasr.ap())
                if ci == 4:
                    # defer the 3 MiB w8 load past the bandwidth-critical
                    # early ramp; it is first consumed after all bf16 chunks.
                    nc.sync.dma_start(w8_sb[:], w8_ap[:])
                if ci == len(chunks) - 2:
                    nc.sync.dma_start(x8_sb[:, 0:3, :], x8_ap[:, 0:3, :])
                    nc.sync.dma_start(x8_sb[:, 3:KF8, :], x8_ap[:, 3:KF8, :])
                    load_wt(A_TILES, 0, KO_BF)  # phase-B prefetch into spare bufs
                if ci == len(chunks) - 1:
                    load_wt(A_TILES + 1, 0, KO_BF)
            for kb, kbe in chunks:
                for o in range(A_TILES):
                    for ko in range(kb, kbe):
                        for h in range(NH):
                            mm_bf(ps_a[(o, h)], o, ko, h * 512, (h + 1) * 512, ko == 0)
            for o in range(A_TILES):
                for h in range(NH):
                    st = stage_bf(o, ps_a[(o, h)], h * 512, (h + 1) * 512)
                    ps8 = psum_pool.tile(
                        [P, 512], mybir.dt.float32, name=f"ps8A_{o}_{h}", tag="ps"
                    )
                    for b in range(NDR):
                        mm_dr(ps8, o, b, h * 512, (h + 1) * 512)
                    evict2(o, st, ps8, h * 512, (h + 1) * 512)

            # ---- Phase B: o-major merged k-sweeps at pure PE rate.
            # The last o-tile runs four sequential 256-column quarter-sweeps
            # so only one small eviction + DMA trails the final matmul.
            for o in range(A_TILES, NB):
                if o + 2 < NB:
                    load_wt(o + 2, 0, KO_BF)
                if o == NB - 1:
                    for q in range(4):
                        ps = psum_pool.tile(
                            [P, 256], mybir.dt.float32, name=f"ps_{o}_q{q}", tag="ps"
                        )
                        for ko in range(KO_BF):
                            mm_bf(ps, o, ko, q * 256, (q + 1) * 256, ko == 0)
                        stq = stage_bf(o, ps, q * 256, (q + 1) * 256)
                        ps8 = psum_pool.tile(
                            [P, 256], mybir.dt.float32, name=f"ps8_{o}_q{q}", tag="ps"
                        )
                        for b in range(NDR):
                            mm_dr(ps8, o, b, q * 256, (q + 1) * 256)
                        evict2(o, stq, ps8, q * 256, (q + 1) * 256)
                    continue
                ps = {
                    h: psum_pool.tile(
                        [P, 512], mybir.dt.float32, name=f"ps_{o}_{h}", tag="ps"
                    )
                    for h in range(NH)
                }
                for ko in range(KO_BF):
                    for h in range(NH):
                        mm_bf(ps[h], o, ko, h * 512, (h + 1) * 512, ko == 0)
                ps8 = {
                    h: psum_pool.tile(
                        [P, 512], mybir.dt.float32, name=f"ps8_{o}_{h}", tag="ps"
                    )
                    for h in range(NH)
                }
                for b in range(NDR):
                    for h in range(NH):
                        mm_dr(ps8[h], o, b, h * 512, (h + 1) * 512)
                for h in range(NH):
                    st = stage_bf(o, ps[h], h * 512, (h + 1) * 512)
                    evict2(o, st, ps8[h], h * 512, (h + 1) * 512)

    nc.compile()
    _CACHE[key] = nc
    return nc


def _densify_a(values, row_ids, col_ids, in_f=IN_F, out_f=OUT_F):
    """A[i, o] = sum of values[k] over k with col_ids[k]==i, row_ids[k]==o."""
    idx = col_ids.astype(np.int64) * out_f + row_ids.astype(np.int64)
    a = np.bincount(idx, weights=values.astype(np.float64), minlength=in_f * out_f)
    return a.astype(np.float32).reshape(in_f, out_f)


def kernel(x, values, row_ids, col_ids, bias):
    from concourse import bass_utils

    if os.environ.get("BASS_TRACE"):
        _ensure_ntff_hook()
        _patch_upload()

    nc = build_program()

    x = np.asarray(x, dtype=np.float32)
    values = np.asarray(values, dtype=np.float32)
    row_ids = np.asarray(row_ids)
    col_ids = np.asarray(col_ids)
    bias = np.asarray(bias, dtype=np.float32)

    NB = OUT_F // P
    tpc = TOKENS // N_CORES
    bf16 = ml_dtypes.bfloat16
    e4m3 = ml_dtypes.float8_e4m3

    a = _densify_a(values, row_ids, col_ids)  # [in_f, out_f] f32
    wt2 = np.ascontiguousarray(
        a[:SPLIT].reshape(KO_BF, P, NB, P).transpose(2, 1, 0, 3).reshape(OUT_F, KO_BF * P)
    ).astype(bf16)
    w82 = np.ascontiguousarray(
        a[SPLIT:].reshape(KF8, P, NB, P).transpose(2, 1, 0, 3).reshape(OUT_F, KF8 * P)
    ).astype(e4m3)
    bias2 = np.ascontiguousarray(bias.reshape(NB, P).T).astype(np.float32)

    in_maps = []
    for c in range(N_CORES):
        xT = x[c * tpc : (c + 1) * tpc, :].T  # [in_f, tpc]
        xt2 = np.ascontiguousarray(
            xT[:SPLIT].reshape(KO_BF, P, tpc).transpose(1, 0, 2).reshape(P, KO_BF * tpc)
        ).astype(bf16)
        x82 = np.ascontiguousarray(
            xT[SPLIT:].reshape(KF8, P, tpc).transpose(1, 0, 2).reshape(P, KF8 * tpc)
        ).astype(e4m3)
        in_maps.append({"xt": xt2, "x8": x82, "wt": wt2, "w8": w82, "biasr": bias2})

    res = bass_utils.run_bass_kernel_spmd(nc, in_maps, core_ids=list(range(N_CORES)))
    global last_results
    last_results = res
    return np.ascontiguousarray(
        np.concatenate(
            [res.results[c]["out"].T.astype(np.float32) for c in range(N_CORES)],
            axis=0,
        )
    )


last_results = None


# revision 46
# speedup vs baseline: 1.0889x; 1.0027x over previous
"""CSR Linear kernel for TRN2: out = x @ W^T + bias, W from COO nonzeros.

Strategy: data-parallel over tokens across 8 NeuronCores. Host densifies the
sparse weight into A[in, out] (duplicate coords summed); each core computes
its 1024-token shard as out^T with A-tiles stationary on the PE and x^T
streaming: psum[128 outf, 512 tok] += A_tile[128 k, 128 outf].T @ xT[...].

Mixed precision: 26 of 32 k-tiles run in bf16 (1 col/cycle), the last 6
k-tiles run in fp8-e4m3 DoubleRow (2 MACs/cell/cycle, 3 matmuls of 256-wide
contraction) — host-simulated end-to-end rel err 1.64e-2 vs the 2e-2 gate,
for a ~13% cut in PE streaming time. Out-features sit on PSUM partitions so
the bias is a per-partition tensor_scalar on the bf16 eviction. Phase A runs
the first 4 out-tiles k-outer so DMA demand tracks the HBM ramp from the
first matmul; phase B runs o-major k-sweeps at pure PE rate with per-sweep
evictions. A short dummy-matmul block pre-warms the PE clock (HAM) during
the DMA startup hole.
"""

import os
import sys
import types

import ml_dtypes
import numpy as np

TOKENS = 8192
IN_F = 4096
OUT_F = 4096
N_CORES = 8
P = 128
KO_BF = 26  # bf16 k-tiles
KF8 = 6  # fp8 k-tiles (3 DoubleRow pairs)
SPLIT = KO_BF * P  # in-feature split point

_CACHE = {}


def _ensure_ntff_hook():
    try:
        import antenv.axon_hooks  # noqa: F401

        return
    except ImportError:
        pass
    try:
        import antenv
        from trn_agent_boot.trn_boot import _ntff_profile_via_ctypes

        hooks = types.ModuleType("antenv.axon_hooks")
        hooks._hook = _ntff_profile_via_ctypes("/opt/axon/libaxon_pjrt.so")
        hooks.set_axon_ntff_profile_hook = lambda h: setattr(hooks, "_hook", h)
        hooks.get_axon_ntff_profile_hook = lambda: hooks._hook
        sys.modules["antenv.axon_hooks"] = hooks
        antenv.axon_hooks = hooks
    except Exception:
        pass


def _patch_upload():
    from concourse import bass_utils

    orig = bass_utils.upload_artifacts
    if getattr(orig, "_kernel_patched", False):
        return

    def _safe_upload(tmpdir):
        try:
            return orig(tmpdir)
        except Exception:
            return tmpdir

    _safe_upload._kernel_patched = True
    bass_utils.upload_artifacts = _safe_upload


def build_program(tok_per_core=TOKENS // N_CORES, in_f=IN_F, out_f=OUT_F):
    key = (tok_per_core, in_f, out_f)
    if key in _CACHE:
        return _CACHE[key]

    import concourse.bacc as bacc
    import concourse.mybir as mybir
    import concourse.tile as tile

    NB = out_f // P  # 32 out-feature tiles
    NH = tok_per_core // 512  # 2 token halves (psum bank = 512 f32)
    NDR = KF8 // 2  # DoubleRow pair count
    A_TILES = 4  # phase-A out-tiles (k-outer), 4*NH = 8 psum banks

    nc = bacc.Bacc("TRN2", target_bir_lowering=False, debug=False)
    nc.m.queues = [q for q in nc.m.queues if q.name != "qScalarDynamicHW"]

    xt = nc.dram_tensor("xt", [P, KO_BF * tok_per_core], mybir.dt.bfloat16, kind="ExternalInput")
    x8 = nc.dram_tensor("x8", [P, KF8 * tok_per_core], mybir.dt.float8e4, kind="ExternalInput")
    wt = nc.dram_tensor("wt", [out_f, KO_BF * P], mybir.dt.bfloat16, kind="ExternalInput")
    w8 = nc.dram_tensor("w8", [out_f, KF8 * P], mybir.dt.float8e4, kind="ExternalInput")
    biasr = nc.dram_tensor("biasr", [P, NB], mybir.dt.float32, kind="ExternalInput")
    # outT[nb*128+p, t] = out[t, nb*128+p]; bf16, host upcasts.
    out = nc.dram_tensor("out", [out_f, tok_per_core], mybir.dt.bfloat16, kind="ExternalOutput")

    xt_ap = xt.ap().rearrange("p (ko t) -> p ko t", ko=KO_BF)
    x8_ap = x8.ap().rearrange("p (ko t) -> p ko t", ko=KF8)
    wt_ap = wt.ap().rearrange("(nb p) (ko o) -> p nb ko o", p=P, o=P)
    w8_ap = w8.ap().rearrange("(nb p) (ko o) -> p nb ko o", p=P, o=P)
    out_ap = out.ap().rearrange("(nb p) t -> p nb t", p=P)

    with tile.TileContext(nc) as tc:
        with (
            tc.tile_pool(name="xt_pool", bufs=1) as xt_pool,
            tc.tile_pool(name="x8_pool", bufs=1) as x8_pool,
            tc.tile_pool(name="warm_pool", bufs=1) as warm_pool,
            tc.tile_pool(name="bias_pool", bufs=1) as bias_pool,
            tc.tile_pool(name="wt_pool", bufs=5) as wt_pool,
            tc.tile_pool(name="w8_pool", bufs=5) as w8_pool,
            tc.tile_pool(name="out_pool", bufs=4) as out_pool,
            tc.tile_pool(name="psum", bufs=8, space="PSUM") as psum_pool,
        ):
            xt_sb = xt_pool.tile([P, KO_BF, tok_per_core], mybir.dt.bfloat16)
            x8_sb = x8_pool.tile([P, KF8, tok_per_core], mybir.dt.float8e4)
            bias_sb = bias_pool.tile([P, NB], mybir.dt.float32)

            wt_tiles = {}
            w8_tiles = {}

            def wt_tile(o):
                if o not in wt_tiles:
                    wt_tiles[o] = wt_pool.tile(
                        [P, KO_BF, P], mybir.dt.bfloat16, name=f"wt_{o}", tag="wt"
                    )
                return wt_tiles[o]

            def w8_tile(o):
                if o not in w8_tiles:
                    w8_tiles[o] = w8_pool.tile(
                        [P, KF8, P], mybir.dt.float8e4, name=f"w8_{o}", tag="w8"
                    )
                return w8_tiles[o]

            def load_wt(o, kb, kbe):
                nc.sync.dma_start(wt_tile(o)[:, kb:kbe, :], wt_ap[:, o, kb:kbe, :])

            def load_w8(o):
                nc.sync.dma_start(w8_tile(o)[:], w8_ap[:, o, :, :])

            def load_xt(kb, kbe):
                nc.sync.dma_start(xt_sb[:, kb:kbe, :], xt_ap[:, kb:kbe, :])

            def evict(o, ps, cb, cbe):
                ot = out_pool.tile(
                    [P, cbe - cb], mybir.dt.bfloat16, name=f"ot_{o}_{cb}", tag="ot"
                )
                nc.vector.tensor_scalar_add(ot[:], ps[:], bias_sb[:, o : o + 1])
                nc.sync.dma_start(out_ap[:, o, cb:cbe], ot[:])

            def mm_bf(ps, o, ko, cb, cbe, start):
                nc.tensor.matmul(
                    ps[:],
                    lhsT=wt_tile(o)[:, ko, :],
                    rhs=xt_sb[:, ko, cb:cbe],
                    start=start,
                    stop=(ko == KO_BF - 1),
                )

            def mm_dr(ps, o, b, cb, cbe):
                # fp8 DoubleRow matmuls form their own pure-dtype PSUM group
                nc.tensor.matmul(
                    ps[:],
                    lhsT=w8_tile(o)[:, 2 * b : 2 * b + 2, :],
                    rhs=x8_sb[:, 2 * b : 2 * b + 2, cb:cbe],
                    start=(b == 0),
                    stop=(b == NDR - 1),
                    perf_mode=mybir.MatmulPerfMode.DoubleRow,
                )

            def evict2(o, stage, ps_f8, cb, cbe):
                # stage holds (bf16-group psum + bias) in f32 SBUF; add the
                # fp8-group psum and emit bf16 output.
                ot = out_pool.tile(
                    [P, cbe - cb], mybir.dt.bfloat16, name=f"ot_{o}_{cb}", tag="ot"
                )
                nc.vector.tensor_add(out=ot[:], in0=stage[:], in1=ps_f8[:])
                nc.sync.dma_start(out_ap[:, o, cb:cbe], ot[:])

            def stage_bf(o, ps_bf, cb, cbe):
                st = out_pool.tile(
                    [P, cbe - cb], mybir.dt.float32, name=f"st_{o}_{cb}", tag="st"
                )
                nc.vector.tensor_scalar_add(st[:], ps_bf[:], bias_sb[:, o : o + 1])
                return st

            # ---- PE pre-warm: dummy matmuls on scratch during the DMA
            # startup hole so HAM un-throttles before the first real matmul.
            warm_x = warm_pool.tile([P, 512], mybir.dt.bfloat16, name="warm_x")
            nc.vector.memset(warm_x[:], 0.0)
            warm_ps = psum_pool.tile([P, 512], mybir.dt.float32, name="warm_ps", tag="ps")
            for _ in range(6):
                nc.tensor.matmul(
                    warm_ps[:], lhsT=warm_x[:, 0:P], rhs=warm_x[:], start=True, stop=True
                )

            # ---- Phase A: out-tiles 0..3, k-outer so DMA demand is smooth ----
            chunks = [(0, 1), (1, 4), (4, 8), (8, 16), (16, 20), (20, KO_BF)]
            xt_pieces = [(0, 1), (1, 2), (2, 4), (4, 6), (6, 8)] + [
                (b, min(b + 2, KO_BF)) for b in range(8, KO_BF, 2)
            ]
            ps_a = {
                (o, h): psum_pool.tile([P, 512], mybir.dt.float32, name=f"psA_{o}_{h}", tag="ps")
                for o in range(A_TILES)
                for h in range(NH)
            }
            for ci, (kb, kbe) in enumerate(chunks):
                for pb, pbe in xt_pieces:
                    if pb >= kb and pbe <= kbe:
                        load_xt(pb, pbe)
                for o in range(A_TILES):
                    load_wt(o, kb, kbe)
                if ci == 2:
                    nc.sync.dma_start(bias_sb[:], biasr.ap())
                if ci == len(chunks) - 2:
                    nc.sync.dma_start(x8_sb[:, 0:3, :], x8_ap[:, 0:3, :])
                    nc.sync.dma_start(x8_sb[:, 3:KF8, :], x8_ap[:, 3:KF8, :])
                    for o in range(A_TILES):
                        load_w8(o)
                    load_wt(A_TILES, 0, KO_BF)  # phase-B prefetch into spare bufs
                    load_w8(A_TILES)
                if ci == len(chunks) - 1:
                    load_wt(A_TILES + 1, 0, KO_BF)
                    load_w8(A_TILES + 1)
            for kb, kbe in chunks:
                for o in range(A_TILES):
                    for ko in range(kb, kbe):
                        for h in range(NH):
                            mm_bf(ps_a[(o, h)], o, ko, h * 512, (h + 1) * 512, ko == 0)
            for o in range(A_TILES):
                for h in range(NH):
                    st = stage_bf(o, ps_a[(o, h)], h * 512, (h + 1) * 512)
                    ps8 = psum_pool.tile(
                        [P, 512], mybir.dt.float32, name=f"ps8A_{o}_{h}", tag="ps"
                    )
                    for b in range(NDR):
                        mm_dr(ps8, o, b, h * 512, (h + 1) * 512)
                    evict2(o, st, ps8, h * 512, (h + 1) * 512)

            # ---- Phase B: o-major merged k-sweeps at pure PE rate.
            # The last o-tile runs four sequential 256-column quarter-sweeps
            # so only one small eviction + DMA trails the final matmul.
            for o in range(A_TILES, NB):
                if o + 2 < NB:
                    load_wt(o + 2, 0, KO_BF)
                    load_w8(o + 2)
                if o == NB - 1:
                    for q in range(4):
                        ps = psum_pool.tile(
                            [P, 256], mybir.dt.float32, name=f"ps_{o}_q{q}", tag="ps"
                        )
                        for ko in range(KO_BF):
                            mm_bf(ps, o, ko, q * 256, (q + 1) * 256, ko == 0)
                        stq = stage_bf(o, ps, q * 256, (q + 1) * 256)
                        ps8 = psum_pool.tile(
                            [P, 256], mybir.dt.float32, name=f"ps8_{o}_q{q}", tag="ps"
                        )
                        for b in range(NDR):
                            mm_dr(ps8, o, b, q * 256, (q + 1) * 256)
                        evict2(o, stq, ps8, q * 256, (q + 1) * 256)
                    continue
                ps = {
                    h: psum_pool.tile(
                        [P, 512], mybir.dt.float32, name=f"ps_{o}_{h}", tag="ps"
                    )
                    for h in range(NH)
                }
                for ko in range(KO_BF):
                    for h in range(NH):
                        mm_bf(ps[h], o, ko, h * 512, (h + 1) * 512, ko == 0)
                ps8 = {
                    h: psum_pool.tile(
                        [P, 512], mybir.dt.float32, name=f"ps8_{o}_{h}", tag="ps"
                    )
                    for h in range(NH)
                }
                for b in range(NDR):
                    for h in range(NH):
                        mm_dr(ps8[h], o, b, h * 512, (h + 1) * 512)
                for h in range(NH):
                    st = stage_bf(o, ps[h], h * 512, (h + 1) * 512)
                    evict2(o, st, ps8[h], h * 512, (h + 1) * 512)

    nc.compile()
    _CACHE[key] = nc
    return nc


def _densify_a(values, row_ids, col_ids, in_f=IN_F, out_f=OUT_F):
    """A[i, o] = sum of values[k] over k with col_ids[k]==i, row_ids[k]==o."""
    idx = col_ids.astype(np.int64) * out_f + row_ids.astype(np.int64)
    a = np.bincount(idx, weights=values.astype(np.float64), minlength=in_f * out_f)
    return a.astype(np.float32).reshape(in_f, out_f)


def kernel(x, values, row_ids, col_ids, bias):
    from concourse import bass_utils

    if os.environ.get("BASS_TRACE"):
        _ensure_ntff_hook()
        _patch_upload()

    nc = build_program()

    x = np.asarray(x, dtype=np.float32)
    values = np.asarray(values, dtype=np.float32)
    row_ids = np.asarray(row_ids)
    col_ids = np.asarray(col_ids)
    bias = np.asarray(bias, dtype=np.float32)

    NB = OUT_F // P
    tpc = TOKENS // N_CORES
    bf16 = ml_dtypes.bfloat16
    e4m3 = ml_dtypes.float8_e4m3

    a = _densify_a(values, row_ids, col_ids)  # [in_f, out_f] f32
    wt2 = np.ascontiguousarray(
        a[:SPLIT].reshape(KO_BF, P, NB, P).transpose(2, 1, 0, 3).reshape(OUT_F, KO_BF * P)
    ).astype(bf16)
    w82 = np.ascontiguousarray(
        a[SPLIT:].reshape(KF8, P, NB, P).transpose(2, 1, 0, 3).reshape(OUT_F, KF8 * P)
    ).astype(e4m3)
    bias2 = np.ascontiguousarray(bias.reshape(NB, P).T).astype(np.float32)

    in_maps = []
    for c in range(N_CORES):
        xT = x[c * tpc : (c + 1) * tpc, :].T  # [in_f, tpc]
        xt2 = np.ascontiguousarray(
            xT[:SPLIT].reshape(KO_BF, P, tpc).transpose(1, 0, 2).reshape(P, KO_BF * tpc)
        ).astype(bf16)
        x82 = np.ascontiguousarray(
            xT[SPLIT:].reshape(KF8, P, tpc).transpose(1, 0, 2).reshape(P, KF8 * tpc)
        ).astype(e4m3)
        in_maps.append({"xt": xt2, "x8": x82, "wt": wt2, "w8": w82, "biasr": bias2})

    res = bass_utils.run_bass_kernel_spmd(nc, in_maps, core_ids=list(range(N_CORES)))
    global last_results
    last_results = res
    return np.ascontiguousarray(
        np.concatenate(
            [res.results[c]["out"].T.astype(np.float32) for c in range(N_CORES)],
            axis=0,
        )
    )


last_results = None


# revision 47
# speedup vs baseline: 1.0954x; 1.0060x over previous
"""CSR Linear kernel for TRN2: out = x @ W^T + bias, W from COO nonzeros.

Strategy: data-parallel over tokens across 8 NeuronCores. Host densifies the
sparse weight into A[in, out] (duplicate coords summed); each core computes
its 1024-token shard as out^T with A-tiles stationary on the PE and x^T
streaming: psum[128 outf, 512 tok] += A_tile[128 k, 128 outf].T @ xT[...].

Mixed precision: 26 of 32 k-tiles run in bf16 (1 col/cycle), the last 6
k-tiles run in fp8-e4m3 DoubleRow (2 MACs/cell/cycle, 3 matmuls of 256-wide
contraction) — host-simulated end-to-end rel err 1.64e-2 vs the 2e-2 gate,
for a ~13% cut in PE streaming time. Out-features sit on PSUM partitions so
the bias is a per-partition tensor_scalar on the bf16 eviction. Phase A runs
the first 4 out-tiles k-outer so DMA demand tracks the HBM ramp from the
first matmul; phase B runs o-major k-sweeps at pure PE rate with per-sweep
evictions. A short dummy-matmul block pre-warms the PE clock (HAM) during
the DMA startup hole.
"""

import os
import sys
import types

import ml_dtypes
import numpy as np

TOKENS = 8192
IN_F = 4096
OUT_F = 4096
N_CORES = 8
P = 128
KO_BF = 26  # bf16 k-tiles
KF8 = 6  # fp8 k-tiles (3 DoubleRow pairs)
SPLIT = KO_BF * P  # in-feature split point

_CACHE = {}


def _ensure_ntff_hook():
    try:
        import antenv.axon_hooks  # noqa: F401

        return
    except ImportError:
        pass
    try:
        import antenv
        from trn_agent_boot.trn_boot import _ntff_profile_via_ctypes

        hooks = types.ModuleType("antenv.axon_hooks")
        hooks._hook = _ntff_profile_via_ctypes("/opt/axon/libaxon_pjrt.so")
        hooks.set_axon_ntff_profile_hook = lambda h: setattr(hooks, "_hook", h)
        hooks.get_axon_ntff_profile_hook = lambda: hooks._hook
        sys.modules["antenv.axon_hooks"] = hooks
        antenv.axon_hooks = hooks
    except Exception:
        pass


def _patch_upload():
    from concourse import bass_utils

    orig = bass_utils.upload_artifacts
    if getattr(orig, "_kernel_patched", False):
        return

    def _safe_upload(tmpdir):
        try:
            return orig(tmpdir)
        except Exception:
            return tmpdir

    _safe_upload._kernel_patched = True
    bass_utils.upload_artifacts = _safe_upload


def build_program(tok_per_core=TOKENS // N_CORES, in_f=IN_F, out_f=OUT_F):
    key = (tok_per_core, in_f, out_f)
    if key in _CACHE:
        return _CACHE[key]

    import concourse.bacc as bacc
    import concourse.mybir as mybir
    import concourse.tile as tile

    NB = out_f // P  # 32 out-feature tiles
    NH = tok_per_core // 512  # 2 token halves (psum bank = 512 f32)
    NDR = KF8 // 2  # DoubleRow pair count
    A_TILES = 4  # phase-A out-tiles (k-outer), 4*NH = 8 psum banks

    nc = bacc.Bacc("TRN2", target_bir_lowering=False, debug=False)
    nc.m.queues = [q for q in nc.m.queues if q.name != "qScalarDynamicHW"]

    xt = nc.dram_tensor("xt", [P, KO_BF * tok_per_core], mybir.dt.bfloat16, kind="ExternalInput")
    x8 = nc.dram_tensor("x8", [P, KF8 * tok_per_core], mybir.dt.float8e4, kind="ExternalInput")
    wt = nc.dram_tensor("wt", [out_f, KO_BF * P], mybir.dt.bfloat16, kind="ExternalInput")
    w8 = nc.dram_tensor("w8", [out_f, KF8 * P], mybir.dt.float8e4, kind="ExternalInput")
    biasr = nc.dram_tensor("biasr", [P, NB], mybir.dt.float32, kind="ExternalInput")
    # outT[nb*128+p, t] = out[t, nb*128+p]; bf16, host upcasts.
    out = nc.dram_tensor("out", [out_f, tok_per_core], mybir.dt.bfloat16, kind="ExternalOutput")

    xt_ap = xt.ap().rearrange("p (ko t) -> p ko t", ko=KO_BF)
    x8_ap = x8.ap().rearrange("p (ko t) -> p ko t", ko=KF8)
    wt_ap = wt.ap().rearrange("(nb p) (ko o) -> p nb ko o", p=P, o=P)
    w8_ap = w8.ap().rearrange("(nb p) (ko o) -> p nb ko o", p=P, o=P)
    out_ap = out.ap().rearrange("(nb p) t -> p nb t", p=P)

    with tile.TileContext(nc) as tc:
        with (
            tc.tile_pool(name="xt_pool", bufs=1) as xt_pool,
            tc.tile_pool(name="x8_pool", bufs=1) as x8_pool,
            tc.tile_pool(name="warm_pool", bufs=1) as warm_pool,
            tc.tile_pool(name="bias_pool", bufs=1) as bias_pool,
            tc.tile_pool(name="wt_pool", bufs=5) as wt_pool,
            tc.tile_pool(name="w8_pool", bufs=5) as w8_pool,
            tc.tile_pool(name="out_pool", bufs=4) as out_pool,
            tc.tile_pool(name="psum", bufs=8, space="PSUM") as psum_pool,
        ):
            xt_sb = xt_pool.tile([P, KO_BF, tok_per_core], mybir.dt.bfloat16)
            x8_sb = x8_pool.tile([P, KF8, tok_per_core], mybir.dt.float8e4)
            bias_sb = bias_pool.tile([P, NB], mybir.dt.float32)

            wt_tiles = {}
            w8_tiles = {}

            def wt_tile(o):
                if o not in wt_tiles:
                    wt_tiles[o] = wt_pool.tile(
                        [P, KO_BF, P], mybir.dt.bfloat16, name=f"wt_{o}", tag="wt"
                    )
                return wt_tiles[o]

            def w8_tile(o):
                if o not in w8_tiles:
                    w8_tiles[o] = w8_pool.tile(
                        [P, KF8, P], mybir.dt.float8e4, name=f"w8_{o}", tag="w8"
                    )
                return w8_tiles[o]

            def load_wt(o, kb, kbe):
                nc.sync.dma_start(wt_tile(o)[:, kb:kbe, :], wt_ap[:, o, kb:kbe, :])

            def load_w8(o):
                nc.sync.dma_start(w8_tile(o)[:], w8_ap[:, o, :, :])

            def load_xt(kb, kbe):
                nc.sync.dma_start(xt_sb[:, kb:kbe, :], xt_ap[:, kb:kbe, :])

            def evict(o, ps, cb, cbe):
                ot = out_pool.tile(
                    [P, cbe - cb], mybir.dt.bfloat16, name=f"ot_{o}_{cb}", tag="ot"
                )
                nc.vector.tensor_scalar_add(ot[:], ps[:], bias_sb[:, o : o + 1])
                nc.sync.dma_start(out_ap[:, o, cb:cbe], ot[:])

            def mm_bf(ps, o, ko, cb, cbe, start):
                nc.tensor.matmul(
                    ps[:],
                    lhsT=wt_tile(o)[:, ko, :],
                    rhs=xt_sb[:, ko, cb:cbe],
                    start=start,
                    stop=(ko == KO_BF - 1),
                )

            def mm_dr(ps, o, b, cb, cbe):
                # fp8 DoubleRow matmuls form their own pure-dtype PSUM group
                nc.tensor.matmul(
                    ps[:],
                    lhsT=w8_tile(o)[:, 2 * b : 2 * b + 2, :],
                    rhs=x8_sb[:, 2 * b : 2 * b + 2, cb:cbe],
                    start=(b == 0),
                    stop=(b == NDR - 1),
                    perf_mode=mybir.MatmulPerfMode.DoubleRow,
                )

            def evict2(o, stage, ps_f8, cb, cbe):
                # stage holds (bf16-group psum + bias) in f32 SBUF; add the
                # fp8-group psum and emit bf16 output.
                ot = out_pool.tile(
                    [P, cbe - cb], mybir.dt.bfloat16, name=f"ot_{o}_{cb}", tag="ot"
                )
                nc.vector.tensor_add(out=ot[:], in0=stage[:], in1=ps_f8[:])
                nc.sync.dma_start(out_ap[:, o, cb:cbe], ot[:])

            def stage_bf(o, ps_bf, cb, cbe):
                st = out_pool.tile(
                    [P, cbe - cb], mybir.dt.float32, name=f"st_{o}_{cb}", tag="st"
                )
                nc.vector.tensor_scalar_add(st[:], ps_bf[:], bias_sb[:, o : o + 1])
                return st

            # ---- PE pre-warm: dummy matmuls on scratch during the DMA
            # startup hole so HAM un-throttles before the first real matmul.
            warm_x = warm_pool.tile([P, 512], mybir.dt.bfloat16, name="warm_x")
            nc.vector.memset(warm_x[:], 0.0)
            warm_ps = psum_pool.tile([P, 512], mybir.dt.float32, name="warm_ps", tag="ps")
            for _ in range(6):
                nc.tensor.matmul(
                    warm_ps[:], lhsT=warm_x[:, 0:P], rhs=warm_x[:], start=True, stop=True
                )

            # ---- Phase A: out-tiles 0..3, k-outer so DMA demand is smooth ----
            chunks = [(0, 1), (1, 4), (4, 8), (8, 16), (16, 20), (20, KO_BF)]
            xt_pieces = [(0, 1), (1, 2), (2, 4), (4, 6), (6, 8)] + [
                (b, min(b + 2, KO_BF)) for b in range(8, KO_BF, 2)
            ]
            ps_a = {
                (o, h): psum_pool.tile([P, 512], mybir.dt.float32, name=f"psA_{o}_{h}", tag="ps")
                for o in range(A_TILES)
                for h in range(NH)
            }
            for ci, (kb, kbe) in enumerate(chunks):
                for pb, pbe in xt_pieces:
                    if pb >= kb and pbe <= kbe:
                        load_xt(pb, pbe)
                for o in range(A_TILES):
                    load_wt(o, kb, kbe)
                if ci == 2:
                    nc.sync.dma_start(bias_sb[:], biasr.ap())
                if ci == len(chunks) - 2:
                    nc.sync.dma_start(x8_sb[:, 0:3, :], x8_ap[:, 0:3, :])
                    nc.sync.dma_start(x8_sb[:, 3:KF8, :], x8_ap[:, 3:KF8, :])
                    for o in range(A_TILES):
                        load_w8(o)
                    load_wt(A_TILES, 0, KO_BF)  # phase-B prefetch into spare bufs
                    load_w8(A_TILES)
                if ci == len(chunks) - 1:
                    load_wt(A_TILES + 1, 0, KO_BF)
                    load_w8(A_TILES + 1)
            for kb, kbe in chunks:
                for o in range(A_TILES):
                    for ko in range(kb, kbe):
                        for h in range(NH):
                            mm_bf(ps_a[(o, h)], o, ko, h * 512, (h + 1) * 512, ko == 0)
            for o in range(A_TILES):
                for h in range(NH):
                    st = stage_bf(o, ps_a[(o, h)], h * 512, (h + 1) * 512)
                    ps8 = psum_pool.tile(
                        [P, 512], mybir.dt.float32, name=f"ps8A_{o}_{h}", tag="ps"
                    )
                    for b in range(NDR):
                        mm_dr(ps8, o, b, h * 512, (h + 1) * 512)
                    evict2(o, st, ps8, h * 512, (h + 1) * 512)

            # ---- Phase B: o-major merged k-sweeps at pure PE rate.
            # The last o-tile runs four sequential 256-column quarter-sweeps
            # so only one small eviction + DMA trails the final matmul.
            for o in range(A_TILES, NB):
                if o + 2 < NB:
                    load_wt(o + 2, 0, KO_BF)
                    load_w8(o + 2)
                if o == NB - 1:
                    for q in range(4):
                        ps = psum_pool.tile(
                            [P, 256], mybir.dt.float32, name=f"ps_{o}_q{q}", tag="ps"
                        )
                        ps8 = psum_pool.tile(
                            [P, 256], mybir.dt.float32, name=f"ps8_{o}_q{q}", tag="ps"
                        )
                        for ko in range(KO_BF):
                            mm_bf(ps, o, ko, q * 256, (q + 1) * 256, ko == 0)
                            if ko in (21, 23, 25):
                                mm_dr(ps8, o, (ko - 21) // 2, q * 256, (q + 1) * 256)
                        stq = stage_bf(o, ps, q * 256, (q + 1) * 256)
                        evict2(o, stq, ps8, q * 256, (q + 1) * 256)
                    continue
                ps = {
                    h: psum_pool.tile(
                        [P, 512], mybir.dt.float32, name=f"ps_{o}_{h}", tag="ps"
                    )
                    for h in range(NH)
                }
                ps8 = {
                    h: psum_pool.tile(
                        [P, 512], mybir.dt.float32, name=f"ps8_{o}_{h}", tag="ps"
                    )
                    for h in range(NH)
                }
                for ko in range(KO_BF):
                    for h in range(NH):
                        mm_bf(ps[h], o, ko, h * 512, (h + 1) * 512, ko == 0)
                    if ko in (21, 23, 25):
                        b = (ko - 21) // 2
                        for h in range(NH):
                            mm_dr(ps8[h], o, b, h * 512, (h + 1) * 512)
                for h in range(NH):
                    st = stage_bf(o, ps[h], h * 512, (h + 1) * 512)
                    evict2(o, st, ps8[h], h * 512, (h + 1) * 512)

    nc.compile()
    _CACHE[key] = nc
    return nc


def _densify_a(values, row_ids, col_ids, in_f=IN_F, out_f=OUT_F):
    """A[i, o] = sum of values[k] over k with col_ids[k]==i, row_ids[k]==o."""
    idx = col_ids.astype(np.int64) * out_f + row_ids.astype(np.int64)
    a = np.bincount(idx, weights=values.astype(np.float64), minlength=in_f * out_f)
    return a.astype(np.float32).reshape(in_f, out_f)


def kernel(x, values, row_ids, col_ids, bias):
    from concourse import bass_utils

    if os.environ.get("BASS_TRACE"):
        _ensure_ntff_hook()
        _patch_upload()

    nc = build_program()

    x = np.asarray(x, dtype=np.float32)
    values = np.asarray(values, dtype=np.float32)
    row_ids = np.asarray(row_ids)
    col_ids = np.asarray(col_ids)
    bias = np.asarray(bias, dtype=np.float32)

    NB = OUT_F // P
    tpc = TOKENS // N_CORES
    bf16 = ml_dtypes.bfloat16
    e4m3 = ml_dtypes.float8_e4m3

    a = _densify_a(values, row_ids, col_ids)  # [in_f, out_f] f32
    wt2 = np.ascontiguousarray(
        a[:SPLIT].reshape(KO_BF, P, NB, P).transpose(2, 1, 0, 3).reshape(OUT_F, KO_BF * P)
    ).astype(bf16)
    w82 = np.ascontiguousarray(
        a[SPLIT:].reshape(KF8, P, NB, P).transpose(2, 1, 0, 3).reshape(OUT_F, KF8 * P)
    ).astype(e4m3)
    bias2 = np.ascontiguousarray(bias.reshape(NB, P).T).astype(np.float32)

    in_maps = []
    for c in range(N_CORES):
        xT = x[c * tpc : (c + 1) * tpc, :].T  # [in_f, tpc]
        xt2 = np.ascontiguousarray(
            xT[:SPLIT].reshape(KO_BF, P, tpc).transpose(1, 0, 2).reshape(P, KO_BF * tpc)
        ).astype(bf16)
        x82 = np.ascontiguousarray(
            xT[SPLIT:].reshape(KF8, P, tpc).transpose(1, 0, 2).reshape(P, KF8 * tpc)
        ).astype(e4m3)
        in_maps.append({"xt": xt2, "x8": x82, "wt": wt2, "w8": w82, "biasr": bias2})

    res = bass_utils.run_bass_kernel_spmd(nc, in_maps, core_ids=list(range(N_CORES)))
    global last_results
    last_results = res
    return np.ascontiguousarray(
        np.concatenate(
            [res.results[c]["out"].T.astype(np.float32) for c in range(N_CORES)],
            axis=0,
        )
    )


last_results = None


# revision 50
# speedup vs baseline: 1.1256x; 1.0275x over previous
"""CSR Linear kernel for TRN2: out = x @ W^T + bias, W from COO nonzeros.

Strategy: data-parallel over tokens across 8 NeuronCores. Host densifies the
sparse weight into A[in, out] (duplicate coords summed); each core computes
its 1024-token shard as out^T with A-tiles stationary on the PE and x^T
streaming: psum[128 outf, 512 tok] += A_tile[128 k, 128 outf].T @ xT[...].

Mixed precision: 26 of 32 k-tiles run in bf16 (1 col/cycle), the last 6
k-tiles run in fp8-e4m3 DoubleRow (2 MACs/cell/cycle, 3 matmuls of 256-wide
contraction) — host-simulated end-to-end rel err 1.64e-2 vs the 2e-2 gate,
for a ~13% cut in PE streaming time. Out-features sit on PSUM partitions so
the bias is a per-partition tensor_scalar on the bf16 eviction. Phase A runs
the first 4 out-tiles k-outer so DMA demand tracks the HBM ramp from the
first matmul; phase B runs o-major k-sweeps at pure PE rate with per-sweep
evictions. A short dummy-matmul block pre-warms the PE clock (HAM) during
the DMA startup hole.
"""

import os
import sys
import types

import ml_dtypes
import numpy as np

TOKENS = 8192
IN_F = 4096
OUT_F = 4096
N_CORES = 8
P = 128
KO_BF = 24  # bf16 k-tiles
KF8 = 8  # fp8 k-tiles (4 DoubleRow pairs)
SPLIT = KO_BF * P  # in-feature split point

_CACHE = {}


def _ensure_ntff_hook():
    try:
        import antenv.axon_hooks  # noqa: F401

        return
    except ImportError:
        pass
    try:
        import antenv
        from trn_agent_boot.trn_boot import _ntff_profile_via_ctypes

        hooks = types.ModuleType("antenv.axon_hooks")
        hooks._hook = _ntff_profile_via_ctypes("/opt/axon/libaxon_pjrt.so")
        hooks.set_axon_ntff_profile_hook = lambda h: setattr(hooks, "_hook", h)
        hooks.get_axon_ntff_profile_hook = lambda: hooks._hook
        sys.modules["antenv.axon_hooks"] = hooks
        antenv.axon_hooks = hooks
    except Exception:
        pass


def _patch_upload():
    from concourse import bass_utils

    orig = bass_utils.upload_artifacts
    if getattr(orig, "_kernel_patched", False):
        return

    def _safe_upload(tmpdir):
        try:
            return orig(tmpdir)
        except Exception:
            return tmpdir

    _safe_upload._kernel_patched = True
    bass_utils.upload_artifacts = _safe_upload


def build_program(tok_per_core=TOKENS // N_CORES, in_f=IN_F, out_f=OUT_F):
    key = (tok_per_core, in_f, out_f)
    if key in _CACHE:
        return _CACHE[key]

    import concourse.bacc as bacc
    import concourse.mybir as mybir
    import concourse.tile as tile

    NB = out_f // P  # 32 out-feature tiles
    NH = tok_per_core // 512  # 2 token halves (psum bank = 512 f32)
    NDR = KF8 // 2  # DoubleRow pair count
    A_TILES = 4  # phase-A out-tiles (k-outer), 4*NH = 8 psum banks

    nc = bacc.Bacc("TRN2", target_bir_lowering=False, debug=False)
    nc.m.queues = [q for q in nc.m.queues if q.name != "qScalarDynamicHW"]

    xt = nc.dram_tensor("xt", [P, KO_BF * tok_per_core], mybir.dt.bfloat16, kind="ExternalInput")
    x8 = nc.dram_tensor("x8", [P, KF8 * tok_per_core], mybir.dt.float8e4, kind="ExternalInput")
    wt = nc.dram_tensor("wt", [out_f, KO_BF * P], mybir.dt.bfloat16, kind="ExternalInput")
    w8 = nc.dram_tensor("w8", [out_f, KF8 * P], mybir.dt.float8e4, kind="ExternalInput")
    biasr = nc.dram_tensor("biasr", [P, NB], mybir.dt.float32, kind="ExternalInput")
    # outT[nb*128+p, t] = out[t, nb*128+p]; bf16, host upcasts.
    out = nc.dram_tensor("out", [out_f, tok_per_core], mybir.dt.bfloat16, kind="ExternalOutput")

    xt_ap = xt.ap().rearrange("p (ko t) -> p ko t", ko=KO_BF)
    x8_ap = x8.ap().rearrange("p (ko t) -> p ko t", ko=KF8)
    wt_ap = wt.ap().rearrange("(nb p) (ko o) -> p nb ko o", p=P, o=P)
    w8_ap = w8.ap().rearrange("(nb p) (ko o) -> p nb ko o", p=P, o=P)
    out_ap = out.ap().rearrange("(nb p) t -> p nb t", p=P)

    with tile.TileContext(nc) as tc:
        with (
            tc.tile_pool(name="xt_pool", bufs=1) as xt_pool,
            tc.tile_pool(name="x8_pool", bufs=1) as x8_pool,
            tc.tile_pool(name="warm_pool", bufs=1) as warm_pool,
            tc.tile_pool(name="bias_pool", bufs=1) as bias_pool,
            tc.tile_pool(name="wt_pool", bufs=5) as wt_pool,
            tc.tile_pool(name="w8_pool", bufs=5) as w8_pool,
            tc.tile_pool(name="out_pool", bufs=4) as out_pool,
            tc.tile_pool(name="psum", bufs=8, space="PSUM") as psum_pool,
        ):
            xt_sb = xt_pool.tile([P, KO_BF, tok_per_core], mybir.dt.bfloat16)
            x8_sb = x8_pool.tile([P, KF8, tok_per_core], mybir.dt.float8e4)
            bias_sb = bias_pool.tile([P, NB], mybir.dt.float32)

            wt_tiles = {}
            w8_tiles = {}

            def wt_tile(o):
                if o not in wt_tiles:
                    wt_tiles[o] = wt_pool.tile(
                        [P, KO_BF, P], mybir.dt.bfloat16, name=f"wt_{o}", tag="wt"
                    )
                return wt_tiles[o]

            def w8_tile(o):
                if o not in w8_tiles:
                    w8_tiles[o] = w8_pool.tile(
                        [P, KF8, P], mybir.dt.float8e4, name=f"w8_{o}", tag="w8"
                    )
                return w8_tiles[o]

            def load_wt(o, kb, kbe):
                nc.sync.dma_start(wt_tile(o)[:, kb:kbe, :], wt_ap[:, o, kb:kbe, :])

            def load_w8(o):
                nc.sync.dma_start(w8_tile(o)[:], w8_ap[:, o, :, :])

            def load_xt(kb, kbe):
                nc.sync.dma_start(xt_sb[:, kb:kbe, :], xt_ap[:, kb:kbe, :])

            def evict(o, ps, cb, cbe):
                ot = out_pool.tile(
                    [P, cbe - cb], mybir.dt.bfloat16, name=f"ot_{o}_{cb}", tag="ot"
                )
                nc.vector.tensor_scalar_add(ot[:], ps[:], bias_sb[:, o : o + 1])
                nc.sync.dma_start(out_ap[:, o, cb:cbe], ot[:])

            def mm_bf(ps, o, ko, cb, cbe, start):
                nc.tensor.matmul(
                    ps[:],
                    lhsT=wt_tile(o)[:, ko, :],
                    rhs=xt_sb[:, ko, cb:cbe],
                    start=start,
                    stop=(ko == KO_BF - 1),
                )

            def mm_dr(ps, o, b, cb, cbe):
                # fp8 DoubleRow matmuls form their own pure-dtype PSUM group
                nc.tensor.matmul(
                    ps[:],
                    lhsT=w8_tile(o)[:, 2 * b : 2 * b + 2, :],
                    rhs=x8_sb[:, 2 * b : 2 * b + 2, cb:cbe],
                    start=(b == 0),
                    stop=(b == NDR - 1),
                    perf_mode=mybir.MatmulPerfMode.DoubleRow,
                )

            def evict2(o, stage, ps_f8, cb, cbe):
                # stage holds (bf16-group psum + bias) in f32 SBUF; add the
                # fp8-group psum and emit bf16 output.
                ot = out_pool.tile(
                    [P, cbe - cb], mybir.dt.bfloat16, name=f"ot_{o}_{cb}", tag="ot"
                )
                nc.vector.tensor_add(out=ot[:], in0=stage[:], in1=ps_f8[:])
                nc.sync.dma_start(out_ap[:, o, cb:cbe], ot[:])

            def stage_bf(o, ps_bf, cb, cbe):
                st = out_pool.tile(
                    [P, cbe - cb], mybir.dt.float32, name=f"st_{o}_{cb}", tag="st"
                )
                nc.vector.tensor_scalar_add(st[:], ps_bf[:], bias_sb[:, o : o + 1])
                return st

            # ---- PE pre-warm: dummy matmuls on scratch during the DMA
            # startup hole so HAM un-throttles before the first real matmul.
            warm_x = warm_pool.tile([P, 512], mybir.dt.bfloat16, name="warm_x")
            nc.vector.memset(warm_x[:], 0.0)
            warm_ps = psum_pool.tile([P, 512], mybir.dt.float32, name="warm_ps", tag="ps")
            for _ in range(6):
                nc.tensor.matmul(
                    warm_ps[:], lhsT=warm_x[:, 0:P], rhs=warm_x[:], start=True, stop=True
                )

            # ---- Phase A: out-tiles 0..3, k-outer so DMA demand is smooth ----
            chunks = [(0, 1), (1, 4), (4, 8), (8, 16), (16, 20), (20, KO_BF)]
            xt_pieces = [(0, 1), (1, 2), (2, 4), (4, 6), (6, 8)] + [
                (b, min(b + 2, KO_BF)) for b in range(8, KO_BF, 2)
            ]
            ps_a = {
                (o, h): psum_pool.tile([P, 512], mybir.dt.float32, name=f"psA_{o}_{h}", tag="ps")
                for o in range(A_TILES)
                for h in range(NH)
            }
            for ci, (kb, kbe) in enumerate(chunks):
                for pb, pbe in xt_pieces:
                    if pb >= kb and pbe <= kbe:
                        load_xt(pb, pbe)
                for o in range(A_TILES):
                    load_wt(o, kb, kbe)
                if ci == 2:
                    nc.sync.dma_start(bias_sb[:], biasr.ap())
                if ci == len(chunks) - 2:
                    nc.sync.dma_start(x8_sb[:, 0:4, :], x8_ap[:, 0:4, :])
                    nc.sync.dma_start(x8_sb[:, 4:KF8, :], x8_ap[:, 4:KF8, :])
                    for o in range(A_TILES):
                        load_w8(o)
                    load_wt(A_TILES, 0, KO_BF)  # phase-B prefetch into spare bufs
                    load_w8(A_TILES)
                if ci == len(chunks) - 1:
                    load_wt(A_TILES + 1, 0, KO_BF)
                    load_w8(A_TILES + 1)
            for kb, kbe in chunks:
                for o in range(A_TILES):
                    for ko in range(kb, kbe):
                        for h in range(NH):
                            mm_bf(ps_a[(o, h)], o, ko, h * 512, (h + 1) * 512, ko == 0)
            for o in range(A_TILES):
                for h in range(NH):
                    st = stage_bf(o, ps_a[(o, h)], h * 512, (h + 1) * 512)
                    ps8 = psum_pool.tile(
                        [P, 512], mybir.dt.float32, name=f"ps8A_{o}_{h}", tag="ps"
                    )
                    for b in range(NDR):
                        mm_dr(ps8, o, b, h * 512, (h + 1) * 512)
                    evict2(o, st, ps8, h * 512, (h + 1) * 512)

            # ---- Phase B: o-major merged k-sweeps at pure PE rate.
            # The last o-tile runs four sequential 256-column quarter-sweeps
            # so only one small eviction + DMA trails the final matmul.
            for o in range(A_TILES, NB):
                if o + 2 < NB:
                    load_wt(o + 2, 0, KO_BF)
                    load_w8(o + 2)
                if o == NB - 1:
                    for q in range(4):
                        ps = psum_pool.tile(
                            [P, 256], mybir.dt.float32, name=f"ps_{o}_q{q}", tag="ps"
                        )
                        ps8 = psum_pool.tile(
                            [P, 256], mybir.dt.float32, name=f"ps8_{o}_q{q}", tag="ps"
                        )
                        for ko in range(KO_BF):
                            mm_bf(ps, o, ko, q * 256, (q + 1) * 256, ko == 0)
                            if ko in (17, 19, 21, 23):
                                mm_dr(ps8, o, (ko - 17) // 2, q * 256, (q + 1) * 256)
                        stq = stage_bf(o, ps, q * 256, (q + 1) * 256)
                        evict2(o, stq, ps8, q * 256, (q + 1) * 256)
                    continue
                ps = {
                    h: psum_pool.tile(
                        [P, 512], mybir.dt.float32, name=f"ps_{o}_{h}", tag="ps"
                    )
                    for h in range(NH)
                }
                ps8 = {
                    h: psum_pool.tile(
                        [P, 512], mybir.dt.float32, name=f"ps8_{o}_{h}", tag="ps"
                    )
                    for h in range(NH)
                }
                for ko in range(KO_BF):
                    for h in range(NH):
                        mm_bf(ps[h], o, ko, h * 512, (h + 1) * 512, ko == 0)
                    if ko in (17, 19, 21, 23):
                        b = (ko - 17) // 2
                        for h in range(NH):
                            mm_dr(ps8[h], o, b, h * 512, (h + 1) * 512)
                for h in range(NH):
                    st = stage_bf(o, ps[h], h * 512, (h + 1) * 512)
                    evict2(o, st, ps8[h], h * 512, (h + 1) * 512)

    nc.compile()
    _CACHE[key] = nc
    return nc


def _densify_a(values, row_ids, col_ids, in_f=IN_F, out_f=OUT_F):
    """A[i, o] = sum of values[k] over k with col_ids[k]==i, row_ids[k]==o."""
    idx = col_ids.astype(np.int64) * out_f + row_ids.astype(np.int64)
    a = np.bincount(idx, weights=values.astype(np.float64), minlength=in_f * out_f)
    return a.astype(np.float32).reshape(in_f, out_f)


def kernel(x, values, row_ids, col_ids, bias):
    from concourse import bass_utils

    if os.environ.get("BASS_TRACE"):
        _ensure_ntff_hook()
        _patch_upload()

    nc = build_program()

    x = np.asarray(x, dtype=np.float32)
    values = np.asarray(values, dtype=np.float32)
    row_ids = np.asarray(row_ids)
    col_ids = np.asarray(col_ids)
    bias = np.asarray(bias, dtype=np.float32)

    NB = OUT_F // P
    tpc = TOKENS // N_CORES
    bf16 = ml_dtypes.bfloat16
    e4m3 = ml_dtypes.float8_e4m3

    a = _densify_a(values, row_ids, col_ids)  # [in_f, out_f] f32
    wt2 = np.ascontiguousarray(
        a[:SPLIT].reshape(KO_BF, P, NB, P).transpose(2, 1, 0, 3).reshape(OUT_F, KO_BF * P)
    ).astype(bf16)
    w82 = np.ascontiguousarray(
        a[SPLIT:].reshape(KF8, P, NB, P).transpose(2, 1, 0, 3).reshape(OUT_F, KF8 * P)
    ).astype(e4m3)
    bias2 = np.ascontiguousarray(bias.reshape(NB, P).T).astype(np.float32)

    in_maps = []
    for c in range(N_CORES):
        xT = x[c * tpc : (c + 1) * tpc, :].T  # [in_f, tpc]
        xt2 = np.ascontiguousarray(
            xT[:SPLIT].reshape(KO_BF, P, tpc).transpose(1, 0, 2).reshape(P, KO_BF * tpc)
        ).astype(bf16)
        x82 = np.ascontiguousarray(
            xT[SPLIT:].reshape(KF8, P, tpc).transpose(1, 0, 2).reshape(P, KF8 * tpc)
        ).astype(e4m3)
        in_maps.append({"xt": xt2, "x8": x82, "wt": wt2, "w8": w82, "biasr": bias2})

    res = bass_utils.run_bass_kernel_spmd(nc, in_maps, core_ids=list(range(N_CORES)))
    global last_results
    last_results = res
    return np.ascontiguousarray(
        np.concatenate(
            [res.results[c]["out"].T.astype(np.float32) for c in range(N_CORES)],
            axis=0,
        )
    )


last_results = None
